# revision 13
# baseline (speedup 1.0000x reference)
"""Distributed Trainium2 kernel for a single-step attention decoder.

Bahdanau attention + single-step GRU + vocab projection with log-softmax,
SPMD across 8 NeuronCores:
  - batch-parallel (8 batches/core) for attention + GRU
  - vocab-sharded output projection with all-gathered max/sum normalizer

Self-contained: hardcodes all shapes; builds/compiles the Bass graph once and
caches it at module level.
"""

import os
import sys

sys.path.insert(0, "/opt/trn_rl_repo")

import numpy as np
import ml_dtypes

import concourse.bass as bass  # noqa: F401
import concourse.bacc as bacc
import concourse.mybir as mybir
import concourse.tile as tile
import concourse.bass_utils as bass_utils
from concourse import masks

BF16 = ml_dtypes.bfloat16
F32 = mybir.dt.float32
BF = mybir.dt.bfloat16

NCORES = 8
B, NK, HID, EMB, KEY, VOCAB = 64, 2048, 1024, 512, 512, 50257
BL = B // NCORES            # 8 batches per core
H3 = 3 * HID                # 3072
VSH = 6283                  # vocab rows per core (8*6283 = 50264 >= 50257)
VPAD = 6656                 # per-core padded to 13*512
NVC = VPAD // 512           # 13 vocab chunks
NEG = -1.0e9

ACT = mybir.ActivationFunctionType
ALU = mybir.AluOpType
AX = mybir.AxisListType


def _build(level=9):
    nc = bacc.Bacc("TRN2", target_bir_lowering=False, debug=False,
                   num_devices=NCORES)

    # ---- I/O ----
    annT_h = nc.dram_tensor("annT", [BL, KEY, NK], BF, kind="ExternalInput")
    annN_h = nc.dram_tensor("annN", [BL, NK, KEY], BF, kind="ExternalInput")
    hidT_h = nc.dram_tensor("hidT", [HID, BL], BF, kind="ExternalInput")
    hidn_h = nc.dram_tensor("hidn", [BL, HID], F32, kind="ExternalInput")
    embT_h = nc.dram_tensor("embT", [EMB, BL], BF, kind="ExternalInput")
    wk_h = nc.dram_tensor("wk", [KEY, HID], BF, kind="ExternalInput")
    wq_h = nc.dram_tensor("wq", [HID, HID], BF, kind="ExternalInput")
    bk_h = nc.dram_tensor("bk", [HID, 1], F32, kind="ExternalInput")
    v_h = nc.dram_tensor("vat", [HID, 1], BF, kind="ExternalInput")
    wihT_h = nc.dram_tensor("wihT", [HID, H3], BF, kind="ExternalInput")
    whhT_h = nc.dram_tensor("whhT", [HID, H3], BF, kind="ExternalInput")
    bih_h = nc.dram_tensor("bih", [1, H3], BF, kind="ExternalInput")
    bhh_h = nc.dram_tensor("bhh", [1, H3], BF, kind="ExternalInput")
    woutT_h = nc.dram_tensor("woutT", [HID, VPAD], BF, kind="ExternalInput")
    bout_h = nc.dram_tensor("bout", [1, VPAD], BF, kind="ExternalInput")

    out_logp = nc.dram_tensor("out_logp", [B, VPAD], F32, kind="ExternalOutput")
    out_h = nc.dram_tensor("out_h", [BL, HID], F32, kind="ExternalOutput")
    out_a = nc.dram_tensor("out_a", [BL, NK], F32, kind="ExternalOutput")

    with tile.TileContext(nc, num_cores=NCORES) as tc:
        with (
            tc.tile_pool(name="pconst", bufs=1) as pconst,
            tc.tile_pool(name="pann", bufs=8) as pann,
            tc.tile_pool(name="ptanh", bufs=10) as ptanh,
            tc.tile_pool(name="pstream", bufs=3) as pstream,
            tc.tile_pool(name="pwork", bufs=2) as pwork,
            tc.tile_pool(name="prow", bufs=1) as prow,
            tc.tile_pool(name="pdram", bufs=1, space="DRAM") as pdram,
        ):
            # ---- constants / resident weights ----
            ident = pconst.tile([128, 128], F32)
            masks.make_identity(nc, ident[:])
            ident_bf = pconst.tile([128, 128], BF)
            masks.make_identity(nc, ident_bf[:])
            ones_bf = pconst.tile([1, 128], BF)
            nc.vector.memset(ones_bf[:], 1.0)

            wk_sb = []
            for kc in range(4):
                t = pconst.tile([128, HID], BF, tag=f"wk{kc}")
                nc.sync.dma_start(out=t[:], in_=wk_h.ap()[kc * 128:(kc + 1) * 128, :])
                wk_sb.append(t)

            v_col = pconst.tile([128, 8], BF)      # (p, hc)
            nc.sync.dma_start(
                out=v_col[:].rearrange("p (t o) -> p t o", o=1),
                in_=v_h.ap().rearrange("(t p) o -> p t o", p=128))
            bk_col = pconst.tile([128, 8], F32)
            nc.sync.dma_start(
                out=bk_col[:].rearrange("p (t o) -> p t o", o=1),
                in_=bk_h.ap().rearrange("(t p) o -> p t o", p=128))
            embT_sb = pconst.tile([128, 4 * BL], BF)   # (p, cc*8+b)
            nc.sync.dma_start(
                out=embT_sb[:].rearrange("p (t b) -> p t b", b=BL),
                in_=embT_h.ap().rearrange("(t p) b -> p t b", p=128))
            hidT_sb = pconst.tile([128, 8 * BL], BF)   # (p, ic*8+b)
            nc.sync.dma_start(
                out=hidT_sb[:].rearrange("p (t b) -> p t b", b=BL),
                in_=hidT_h.ap().rearrange("(t p) b -> p t b", p=128))
            hidn_sb = pconst.tile([BL, HID], F32)
            nc.sync.dma_start(out=hidn_sb[:], in_=hidn_h.ap())

            # ---- P0: biasT[:, hc*8+b] = (Q @ Wq).T + bk ----
            # scoped PSUM pool (8 banks) — closed before the main psum pools open
            biasT = pconst.tile([128, 8 * BL], F32)
            with tc.tile_pool(name="pq", bufs=8, space="PSUM") as pq:
                qps = [pq.tile([128, BL], F32, tag="qq", name=f"qq{i}")
                       for i in range(8)]
                for ic in range(8):
                    wqt = pstream.tile([128, HID], BF, tag="wq")
                    nc.sync.dma_start(
                        out=wqt[:], in_=wq_h.ap()[ic * 128:(ic + 1) * 128, :])
                    for hc in range(8):
                        nc.tensor.matmul(
                            qps[hc][:], wqt[:, hc * 128:(hc + 1) * 128],
                            hidT_sb[:, ic * BL:(ic + 1) * BL],
                            start=(ic == 0), stop=(ic == 7))
                for hc in range(8):
                    nc.vector.tensor_scalar_add(
                        biasT[:, hc * BL:(hc + 1) * BL], qps[hc][:],
                        bk_col[:, hc:hc + 1])

            with (
                tc.tile_pool(name="ppre", bufs=2, space="PSUM") as ppre,
                tc.tile_pool(name="pscore", bufs=2, space="PSUM") as pscore,
                tc.tile_pool(name="psmall", bufs=2, space="PSUM") as psmall,
            ):
                # ---- P1-P2 per local batch b ----
                a_nat = pconst.tile([BL, NK], BF)          # attention weights
                nbatch = BL if level >= 2 else 1
                for b in range(nbatch):
                    ann_t = []
                    for kc in range(4):
                        t = pann.tile([128, NK], BF, tag="ann")
                        nc.sync.dma_start(
                            out=t[:],
                            in_=annT_h.ap()[b, kc * 128:(kc + 1) * 128, :])
                        ann_t.append(t)

                    # scores row for this batch (partition 0)
                    row_b = prow.tile([1, NK], F32, tag="rowb")
                    for n4 in range(4):
                        tanh_tiles = []
                        for hc in range(8):
                            pre = ppre.tile([128, 512], F32, tag="pre")
                            for kc in range(4):
                                nc.tensor.matmul(
                                    pre[:], wk_sb[kc][:, hc * 128:(hc + 1) * 128],
                                    ann_t[kc][:, n4 * 512:(n4 + 1) * 512],
                                    start=(kc == 0), stop=(kc == 3))
                            th = ptanh.tile([128, 512], BF, tag="tanh")
                            nc.scalar.activation(
                                th[:], pre[:], ACT.Tanh,
                                bias=biasT[:, hc * BL + b:hc * BL + b + 1],
                                scale=1.0)
                            tanh_tiles.append(th)
                        sps = pscore.tile([1, 512], F32, tag="score")
                        for hc in range(8):
                            nc.tensor.matmul(
                                sps[:], v_col[:, hc:hc + 1], tanh_tiles[hc][:],
                                start=(hc == 0), stop=(hc == 7))
                        nc.vector.tensor_copy(
                            row_b[0:1, n4 * 512:(n4 + 1) * 512], sps[:])

                    # P2: softmax on row_b (all on partition 0)
                    mrow = prow.tile([1, 4], F32, tag="mrow")
                    nc.vector.tensor_reduce(mrow[0:1, 0:1], row_b[:],
                                            axis=AX.X, op=ALU.max)
                    nc.vector.tensor_scalar_mul(mrow[0:1, 1:2], mrow[0:1, 0:1],
                                                -1.0)
                    nc.scalar.activation(row_b[:], row_b[:], ACT.Exp,
                                         bias=mrow[0:1, 1:2], scale=1.0,
                                         accum_out=mrow[0:1, 2:3])
                    nc.vector.reciprocal(mrow[0:1, 3:4], mrow[0:1, 2:3])
                    nc.vector.tensor_scalar_mul(row_b[:], row_b[:],
                                                mrow[0:1, 3:4])
                    nc.sync.dma_start(out=out_a.ap()[b:b + 1, :], in_=row_b[:])
                    a_bf = prow.tile([1, NK], BF, tag="abf")
                    nc.vector.tensor_copy(a_bf[:], row_b[:])
                    nc.sync.dma_start(out=a_nat[b:b + 1, :], in_=a_bf[:])

                # ---- P3: context = a @ ann (natural layout) ----
                # transpose a_nat into per-tile columns [128, BL]
                aT_sb = []
                for t in range(16):
                    tp = psmall.tile([128, BL], BF, tag="smallbf")
                    nc.tensor.transpose(tp[:],
                                        a_nat[0:BL, t * 128:(t + 1) * 128],
                                        ident_bf[0:BL, 0:BL])
                    at = pconst.tile([128, BL], BF, tag=f"aT{t}")
                    nc.vector.tensor_copy(at[:], tp[:])
                    aT_sb.append(at)
                ctx_nat = pconst.tile([BL, KEY], F32)
                for b in range(nbatch):
                    cps = pscore.tile([1, KEY], F32, tag="score")
                    for t in range(16):
                        an = pstream.tile([128, KEY], BF, tag="annN")
                        nc.sync.dma_start(
                            out=an[:],
                            in_=annN_h.ap()[b, t * 128:(t + 1) * 128, :])
                        nc.tensor.matmul(cps[:], aT_sb[t][:, b:b + 1], an[:],
                                         start=(t == 0), stop=(t == 15))
                    crow = pwork.tile([1, KEY], F32, tag="crow")
                    nc.vector.tensor_copy(crow[:], cps[:])
                    nc.sync.dma_start(out=ctx_nat[b:b + 1, :], in_=crow[:])

                if level >= 3:
                    # ctxT chunks [128, BL] for the GRU x.T stationary
                    ctx_bf = pconst.tile([BL, KEY], BF)
                    nc.vector.tensor_copy(ctx_bf[:], ctx_nat[:])
                    ctxT_bf = pconst.tile([128, 4 * BL], BF)
                    for t in range(4):
                        tp = psmall.tile([128, BL], BF, tag="smallbf")
                        nc.tensor.transpose(tp[:],
                                            ctx_bf[0:BL, t * 128:(t + 1) * 128],
                                            ident_bf[0:BL, 0:BL])
                        nc.vector.tensor_copy(
                            ctxT_bf[:, t * BL:(t + 1) * BL], tp[:])

                    # ---- P4: GRU ----
                    gi_sb = pconst.tile([BL, H3], F32)
                    gh_sb = pconst.tile([BL, H3], F32)
                    for which, w_h, b_h, g_sb in (
                        ("ih", wihT_h, bih_h, gi_sb),
                        ("hh", whhT_h, bhh_h, gh_sb),
                    ):
                        for rc in range(6):
                            bt = pstream.tile([1, 512], BF, tag="bg")
                            nc.sync.dma_start(
                                out=bt[:],
                                in_=b_h.ap()[0:1, rc * 512:(rc + 1) * 512])
                            gps = psmall.tile([BL, 512], F32, tag="small")
                            nc.tensor.matmul(gps[:], ones_bf[0:1, 0:BL], bt[:],
                                             start=True, stop=False)
                            for cc in range(8):
                                if which == "ih":
                                    lhsT = (embT_sb[:, cc * BL:(cc + 1) * BL]
                                            if cc < 4 else
                                            ctxT_bf[:, (cc - 4) * BL:(cc - 3) * BL])
                                else:
                                    lhsT = hidT_sb[:, cc * BL:(cc + 1) * BL]
                                wt = pstream.tile([128, 512], BF, tag="wg")
                                nc.sync.dma_start(
                                    out=wt[:],
                                    in_=w_h.ap()[cc * 128:(cc + 1) * 128,
                                                 rc * 512:(rc + 1) * 512])
                                nc.tensor.matmul(gps[:], lhsT, wt[:],
                                                 start=False, stop=(cc == 7))
                            nc.vector.tensor_copy(
                                g_sb[0:BL, rc * 512:(rc + 1) * 512], gps[:])

                    # gates
                    ta = pconst.tile([BL, HID], F32, tag="ta")
                    tb = pconst.tile([BL, HID], F32, tag="tb")
                    n_sb = pconst.tile([BL, HID], F32, tag="ngate")
                    h_sb = pconst.tile([BL, HID], F32, tag="hnew")
                    # r
                    nc.vector.tensor_tensor(out=ta[:], in0=gi_sb[0:BL, 0:HID],
                                            in1=gh_sb[0:BL, 0:HID], op=ALU.add)
                    nc.scalar.activation(tb[:], ta[:], ACT.Sigmoid)
                    # n = tanh(gi_n + r*gh_n)
                    nc.vector.tensor_tensor(out=ta[:], in0=tb[:],
                                            in1=gh_sb[0:BL, 2 * HID:H3],
                                            op=ALU.mult)
                    nc.vector.tensor_tensor(out=ta[:],
                                            in0=gi_sb[0:BL, 2 * HID:H3],
                                            in1=ta[:], op=ALU.add)
                    nc.scalar.activation(n_sb[:], ta[:], ACT.Tanh)
                    # z
                    nc.vector.tensor_tensor(out=ta[:],
                                            in0=gi_sb[0:BL, HID:2 * HID],
                                            in1=gh_sb[0:BL, HID:2 * HID],
                                            op=ALU.add)
                    nc.scalar.activation(tb[:], ta[:], ACT.Sigmoid)
                    # h = n + z*(Q - n)
                    nc.vector.tensor_tensor(out=ta[:], in0=hidn_sb[:],
                                            in1=n_sb[:], op=ALU.subtract)
                    nc.vector.tensor_tensor(out=ta[:], in0=tb[:], in1=ta[:],
                                            op=ALU.mult)
                    nc.vector.tensor_tensor(out=h_sb[:], in0=n_sb[:],
                                            in1=ta[:], op=ALU.add)
                    nc.sync.dma_start(out=out_h.ap()[:, :], in_=h_sb[:])

                if level >= 4:
                    # ---- P5: all-gather h ----
                    hb_in = pdram.tile([BL, HID], F32)
                    hb_out = pdram.tile([B, HID], F32)
                    nc.sync.dma_start(out=hb_in[:], in_=h_sb[:])
                    nc.gpsimd.collective_compute(
                        "AllGather", ALU.bypass,
                        replica_groups=[list(range(NCORES))],
                        ins=[hb_in.opt()], outs=[hb_out.opt()])
                    hall = pconst.tile([B, HID], F32)
                    nc.sync.dma_start(out=hall[:], in_=hb_out[:])
                    hallT = []
                    for hc in range(8):
                        tp = psmall.tile([128, B], F32, tag="small")
                        nc.tensor.transpose(tp[:],
                                            hall[:, hc * 128:(hc + 1) * 128],
                                            ident[0:B, 0:B])
                        ht = pconst.tile([128, B], BF, tag=f"hallT{hc}")
                        nc.vector.tensor_copy(ht[:], tp[:])
                        hallT.append(ht)

                if level >= 5:
                    # ---- P6: logits + local stats ----
                    logits = pconst.tile([B, VPAD], BF)
                    mx_sb = pconst.tile([B, 16], F32, tag="mx")
                    es_sb = pconst.tile([B, 16], F32, tag="es")
                    for vc in range(NVC):
                        bt = pstream.tile([1, 512], BF, tag="bg")
                        nc.sync.dma_start(
                            out=bt[:],
                            in_=bout_h.ap()[0:1, vc * 512:(vc + 1) * 512])
                        lp = ppre.tile([B, 512], F32, tag="pre")
                        nc.tensor.matmul(lp[:], ones_bf[0:1, 0:B], bt[:],
                                         start=True, stop=False)
                        for hc in range(8):
                            wot = pstream.tile([128, 512], BF, tag="wout")
                            nc.sync.dma_start(
                                out=wot[:],
                                in_=woutT_h.ap()[hc * 128:(hc + 1) * 128,
                                                 vc * 512:(vc + 1) * 512])
                            nc.tensor.matmul(lp[:], hallT[hc][:], wot[:],
                                             start=False, stop=(hc == 7))
                        nc.vector.tensor_copy(
                            logits[0:B, vc * 512:(vc + 1) * 512], lp[:])
                        nc.vector.tensor_reduce(mx_sb[0:B, vc:vc + 1], lp[:],
                                                axis=AX.X, op=ALU.max)
                    m_loc = pconst.tile([B, 4], F32, tag="mloc")
                    nc.vector.tensor_reduce(m_loc[0:B, 0:1], mx_sb[0:B, 0:NVC],
                                            axis=AX.X, op=ALU.max)
                    nc.vector.tensor_scalar_mul(m_loc[0:B, 1:2],
                                                m_loc[0:B, 0:1], -1.0)
                    for vc in range(NVC):
                        et = pwork.tile([B, 512], F32, tag="et")
                        nc.scalar.activation(
                            et[:], logits[0:B, vc * 512:(vc + 1) * 512],
                            ACT.Exp, bias=m_loc[0:B, 1:2], scale=1.0,
                            accum_out=es_sb[0:B, vc:vc + 1])
                    nc.vector.tensor_reduce(m_loc[0:B, 2:3], es_sb[0:B, 0:NVC],
                                            axis=AX.X, op=ALU.add)

                    # stats row: [1, 128] = [m_loc | s_loc] across batches
                    stats = pconst.tile([1, 2 * B], F32, tag="stats")
                    nc.sync.dma_start(out=stats[0:1, 0:B], in_=m_loc[0:B, 0:1])
                    nc.sync.dma_start(out=stats[0:1, B:2 * B],
                                      in_=m_loc[0:B, 2:3])
                    st_in = pdram.tile([1, 2 * B], F32)
                    st_out = pdram.tile([NCORES, 2 * B], F32)
                    nc.sync.dma_start(out=st_in[:], in_=stats[:])
                    nc.gpsimd.collective_compute(
                        "AllGather", ALU.bypass,
                        replica_groups=[list(range(NCORES))],
                        ins=[st_in.opt()], outs=[st_out.opt()])
                    statsall = pconst.tile([NCORES, 2 * B], F32, tag="statsall")
                    nc.sync.dma_start(out=statsall[:], in_=st_out[:])

                    # transpose to [B, NCORES]
                    mT = pconst.tile([B, NCORES], F32, tag="mT")
                    sT = pconst.tile([B, NCORES], F32, tag="sT")
                    tpm = psmall.tile([B, NCORES], F32, tag="small")
                    nc.tensor.transpose(tpm[:], statsall[0:NCORES, 0:B],
                                        ident[0:NCORES, 0:NCORES])
                    nc.vector.tensor_copy(mT[:], tpm[:])
                    tps = psmall.tile([B, NCORES], F32, tag="small")
                    nc.tensor.transpose(tps[:], statsall[0:NCORES, B:2 * B],
                                        ident[0:NCORES, 0:NCORES])
                    nc.vector.tensor_copy(sT[:], tps[:])

                    # global normalizer L = m_g + ln(sum_c s_c * exp(m_c - m_g))
                    fin = pconst.tile([B, 8], F32, tag="fin")
                    wrk = pconst.tile([B, NCORES], F32, tag="wrk")
                    nc.vector.tensor_reduce(fin[0:B, 0:1], mT[:], axis=AX.X,
                                            op=ALU.max)
                    nc.vector.tensor_scalar_sub(wrk[:], mT[:], fin[0:B, 0:1])
                    nc.scalar.activation(wrk[:], wrk[:], ACT.Exp)
                    nc.vector.tensor_tensor(out=wrk[:], in0=wrk[:], in1=sT[:],
                                            op=ALU.mult)
                    nc.vector.tensor_reduce(fin[0:B, 1:2], wrk[:], axis=AX.X,
                                            op=ALU.add)
                    nc.scalar.activation(fin[0:B, 2:3], fin[0:B, 1:2], ACT.Ln)
                    nc.vector.tensor_tensor(out=fin[0:B, 3:4],
                                            in0=fin[0:B, 0:1],
                                            in1=fin[0:B, 2:3], op=ALU.add)
                    # logp = logits - L, in f32 staging chunks
                    for vc in range(NVC):
                        lo = pwork.tile([B, 512], F32, tag="lpout")
                        nc.vector.tensor_scalar_sub(
                            lo[:], logits[0:B, vc * 512:(vc + 1) * 512],
                            fin[0:B, 3:4])
                        nc.sync.dma_start(
                            out=out_logp.ap()[:, vc * 512:(vc + 1) * 512],
                            in_=lo[:])

    nc.compile()
    return nc


_NC = None


def _get_nc():
    global _NC
    if _NC is None:
        _NC = _build(int(os.environ.get("KLEVEL", "9")))
    return _NC


def _shard(inputs):
    ids = np.asarray(inputs["input_ids"]).reshape(-1).astype(np.int64)
    emb = np.asarray(inputs["emb_table"], dtype=np.float32)[ids]      # [64, 512]
    hid = np.asarray(inputs["hidden"], dtype=np.float32)[0]           # [64, 1024]
    ann = np.asarray(inputs["annotations"], dtype=np.float32)         # [64, 2048, 512]

    wk_bf = np.asarray(inputs["Wk"], dtype=np.float32).astype(BF16)   # [512, 1024]
    wq_bf = np.asarray(inputs["Wq"], dtype=np.float32).astype(BF16)   # [1024, 1024]
    bk_col = np.ascontiguousarray(
        np.asarray(inputs["bk"], dtype=np.float32).reshape(HID, 1))
    v_bf = np.ascontiguousarray(
        np.asarray(inputs["v_attn"], dtype=np.float32).reshape(HID, 1)).astype(BF16)
    wihT = np.ascontiguousarray(
        np.asarray(inputs["W_ih"], dtype=np.float32).T).astype(BF16)  # [1024, 3072]
    whhT = np.ascontiguousarray(
        np.asarray(inputs["W_hh"], dtype=np.float32).T).astype(BF16)
    bih = np.asarray(inputs["b_ih"], dtype=np.float32).astype(BF16).reshape(1, H3)
    bhh = np.asarray(inputs["b_hh"], dtype=np.float32).astype(BF16).reshape(1, H3)
    wout = np.asarray(inputs["W_out"], dtype=np.float32)              # [50257, 1024]
    bout = np.asarray(inputs["b_out"], dtype=np.float32)

    in_maps = []
    for c in range(NCORES):
        sl = slice(c * BL, (c + 1) * BL)
        ann_bf = ann[sl].astype(BF16)
        annT = np.ascontiguousarray(ann_bf.transpose(0, 2, 1))
        annN = ann_bf
        hidT = np.ascontiguousarray(hid[sl].T).astype(BF16)           # [1024, 8]
        hidn = np.ascontiguousarray(hid[sl])                          # [8, 1024]
        embT = np.ascontiguousarray(emb[sl].T).astype(BF16)           # [512, 8]

        lo = c * VSH
        hi = min(lo + VSH, VOCAB)
        nrow = max(hi - lo, 0)
        woutT = np.zeros((HID, VPAD), dtype=BF16)
        if nrow > 0:
            woutT[:, :nrow] = wout[lo:hi].T.astype(BF16)
        bout_sh = np.full((1, VPAD), NEG, dtype=np.float32)
        if nrow > 0:
            bout_sh[0, :nrow] = bout[lo:hi]
        in_maps.append({
            "annT": annT,
            "annN": annN,
            "hidT": hidT,
            "hidn": hidn,
            "embT": embT,
            "wk": wk_bf,
            "wq": wq_bf,
            "bk": bk_col,
            "vat": v_bf,
            "wihT": wihT,
            "whhT": whhT,
            "bih": bih,
            "bhh": bhh,
            "woutT": woutT,
            "bout": bout_sh.astype(BF16),
        })
    return in_maps


def kernel(trace=False, **inputs):
    nc = _get_nc()
    in_maps = _shard(inputs)
    res = bass_utils.run_bass_kernel_spmd(
        nc, in_maps, core_ids=list(range(NCORES)), trace=trace)
    results = res.results
    logp = np.concatenate(
        [results[c]["out_logp"][:, :VSH] for c in range(NCORES)], axis=1)
    logp = logp[:, :VOCAB].astype(np.float32)
    o = np.concatenate([results[c]["out_h"] for c in range(NCORES)], axis=0)
    a = np.concatenate([results[c]["out_a"] for c in range(NCORES)], axis=0)
    if trace:
        kernel.last_exec_time_ns = res.exec_time_ns
    return logp[None], o[None].astype(np.float32), a.astype(np.float32)


# revision 28
# speedup vs baseline: 1.4078x; 1.4078x over previous
"""Distributed Trainium2 kernel for a single-step attention decoder.

Bahdanau attention + single-step GRU + vocab projection with log-softmax,
SPMD across 8 NeuronCores:
  - batch-parallel (8 batches/core) for attention + GRU
  - vocab-sharded output projection with all-gathered max/sum normalizer

Self-contained: hardcodes all shapes; builds/compiles the Bass graph once and
caches it at module level.
"""

import os
import sys

sys.path.insert(0, "/opt/trn_rl_repo")

import numpy as np
import ml_dtypes

import concourse.bass as bass  # noqa: F401
import concourse.bacc as bacc
import concourse.mybir as mybir
import concourse.tile as tile
import concourse.bass_utils as bass_utils
from concourse import masks

BF16 = ml_dtypes.bfloat16
F32 = mybir.dt.float32
BF = mybir.dt.bfloat16

NCORES = 8
B, NK, HID, EMB, KEY, VOCAB = 64, 2048, 1024, 512, 512, 50257
BL = B // NCORES            # 8 batches per core
H3 = 3 * HID                # 3072
VSH = 6283                  # vocab rows per core (8*6283 = 50264 >= 50257)
VPAD = 6656                 # per-core padded to 13*512
NVC = VPAD // 512           # 13 vocab chunks
NEG = -1.0e9

ACT = mybir.ActivationFunctionType
ALU = mybir.AluOpType
AX = mybir.AxisListType


def _build(level=9):
    nc = bacc.Bacc("TRN2", target_bir_lowering=False, debug=False,
                   num_devices=NCORES)

    # ---- I/O ----
    # annT: transposed [key, nk] per batch; annN: tile-major natural
    # [b][128p][t*512+k] with t the nk-tile index
    annT_h = nc.dram_tensor("annT", [BL, KEY, NK], BF, kind="ExternalInput")
    annN_h = nc.dram_tensor("annN", [BL, 128, 16 * KEY], BF, kind="ExternalInput")
    hidT_h = nc.dram_tensor("hidT", [HID, BL], BF, kind="ExternalInput")
    hidn_h = nc.dram_tensor("hidn", [BL, HID], F32, kind="ExternalInput")
    embT_h = nc.dram_tensor("embT", [EMB, BL], BF, kind="ExternalInput")
    wk_h = nc.dram_tensor("wk", [KEY, HID], BF, kind="ExternalInput")
    wq_h = nc.dram_tensor("wq", [HID, HID], BF, kind="ExternalInput")
    bk_h = nc.dram_tensor("bk", [HID, 1], F32, kind="ExternalInput")
    v_h = nc.dram_tensor("vat", [HID, 1], BF, kind="ExternalInput")
    # tile-major weight layouts: [chunk][128p][sub*512+j]
    wihT_h = nc.dram_tensor("wihT", [6, 128, 8 * 512], BF, kind="ExternalInput")
    whhT_h = nc.dram_tensor("whhT", [6, 128, 8 * 512], BF, kind="ExternalInput")
    bih_h = nc.dram_tensor("bih", [1, H3], BF, kind="ExternalInput")
    bhh_h = nc.dram_tensor("bhh", [1, H3], BF, kind="ExternalInput")
    woutT_h = nc.dram_tensor("woutT", [NVC, 128, 8 * 512], BF, kind="ExternalInput")
    bout_h = nc.dram_tensor("bout", [1, VPAD], BF, kind="ExternalInput")

    out_logp = nc.dram_tensor("out_logp", [B, VPAD], F32, kind="ExternalOutput")
    out_h = nc.dram_tensor("out_h", [BL, HID], F32, kind="ExternalOutput")
    out_a = nc.dram_tensor("out_a", [BL, NK], F32, kind="ExternalOutput")

    with tile.TileContext(nc, num_cores=NCORES) as tc:
        with (
            tc.tile_pool(name="pconst", bufs=1) as pconst,
            tc.tile_pool(name="pann", bufs=8) as pann,
            tc.tile_pool(name="ptanh", bufs=10) as ptanh,
            tc.tile_pool(name="pstream", bufs=3) as pstream,
            tc.tile_pool(name="pwg", bufs=2) as pwg,
            tc.tile_pool(name="pwot", bufs=2) as pwot,
            tc.tile_pool(name="pannN", bufs=2) as pannN,
            tc.tile_pool(name="pwork", bufs=2) as pwork,
            tc.tile_pool(name="prow", bufs=1) as prow,
            tc.tile_pool(name="pdram", bufs=1, space="DRAM") as pdram,
        ):
            # ---- constants / resident weights ----
            ident = pconst.tile([128, 128], F32)
            masks.make_identity(nc, ident[:])
            ident_bf = pconst.tile([128, 128], BF)
            masks.make_identity(nc, ident_bf[:])
            ones_bf = pconst.tile([1, 128], BF)
            nc.vector.memset(ones_bf[:], 1.0)

            wk_sb = []
            for kc in range(4):
                t = pconst.tile([128, HID], BF, tag=f"wk{kc}")
                nc.sync.dma_start(out=t[:], in_=wk_h.ap()[kc * 128:(kc + 1) * 128, :])
                wk_sb.append(t)

            v_col = pconst.tile([128, 8], BF)      # (p, hc)
            nc.sync.dma_start(
                out=v_col[:].rearrange("p (t o) -> p t o", o=1),
                in_=v_h.ap().rearrange("(t p) o -> p t o", p=128))
            bk_col = pconst.tile([128, 8], F32)
            nc.sync.dma_start(
                out=bk_col[:].rearrange("p (t o) -> p t o", o=1),
                in_=bk_h.ap().rearrange("(t p) o -> p t o", p=128))
            embT_sb = pconst.tile([128, 4 * BL], BF)   # (p, cc*8+b)
            nc.sync.dma_start(
                out=embT_sb[:].rearrange("p (t b) -> p t b", b=BL),
                in_=embT_h.ap().rearrange("(t p) b -> p t b", p=128))
            hidT_sb = pconst.tile([128, 8 * BL], BF)   # (p, ic*8+b)
            nc.sync.dma_start(
                out=hidT_sb[:].rearrange("p (t b) -> p t b", b=BL),
                in_=hidT_h.ap().rearrange("(t p) b -> p t b", p=128))
            hidn_sb = pconst.tile([BL, HID], F32)
            nc.sync.dma_start(out=hidn_sb[:], in_=hidn_h.ap())

            # ---- P0: biasT[:, hc*8+b] = (Q @ Wq).T + bk ----
            # scoped PSUM pool (8 banks) — closed before the main psum pools open
            biasT = pconst.tile([128, 8 * BL], F32)
            with tc.tile_pool(name="pq", bufs=8, space="PSUM") as pq:
                qps = [pq.tile([128, BL], F32, tag="qq", name=f"qq{i}")
                       for i in range(8)]
                for ic in range(8):
                    wqt = pstream.tile([128, HID], BF, tag="wq")
                    nc.sync.dma_start(
                        out=wqt[:], in_=wq_h.ap()[ic * 128:(ic + 1) * 128, :])
                    for hc in range(8):
                        nc.tensor.matmul(
                            qps[hc][:], wqt[:, hc * 128:(hc + 1) * 128],
                            hidT_sb[:, ic * BL:(ic + 1) * BL],
                            start=(ic == 0), stop=(ic == 7))
                for hc in range(8):
                    nc.vector.tensor_scalar_add(
                        biasT[:, hc * BL:(hc + 1) * BL], qps[hc][:],
                        bk_col[:, hc:hc + 1])

            with (
                tc.tile_pool(name="ppre", bufs=2, space="PSUM") as ppre,
                tc.tile_pool(name="pscore", bufs=2, space="PSUM") as pscore,
                tc.tile_pool(name="psmall", bufs=2, space="PSUM") as psmall,
            ):
                # ---- P1-P2 per local batch b ----
                a_nat = pconst.tile([BL, NK], BF)          # attention weights
                nbatch = BL if level >= 2 else 1
                for b in range(nbatch):
                    ann_t = []
                    for kc in range(4):
                        t = pann.tile([128, NK], BF, tag="ann")
                        nc.sync.dma_start(
                            out=t[:],
                            in_=annT_h.ap()[b, kc * 128:(kc + 1) * 128, :])
                        ann_t.append(t)

                    # scores row for this batch (partition 0)
                    row_b = prow.tile([1, NK], F32, tag="rowb")
                    for n4 in range(4):
                        tanh_tiles = []
                        for hc in range(8):
                            pre = ppre.tile([128, 512], F32, tag="pre")
                            for kc in range(4):
                                nc.tensor.matmul(
                                    pre[:], wk_sb[kc][:, hc * 128:(hc + 1) * 128],
                                    ann_t[kc][:, n4 * 512:(n4 + 1) * 512],
                                    start=(kc == 0), stop=(kc == 3))
                            th = ptanh.tile([128, 512], BF, tag="tanh")
                            nc.scalar.activation(
                                th[:], pre[:], ACT.Tanh,
                                bias=biasT[:, hc * BL + b:hc * BL + b + 1],
                                scale=1.0)
                            tanh_tiles.append(th)
                        # score reduce: two concurrent col-tile chains
                        sps = pscore.tile([64, 512], F32, tag="score")
                        for k in range(4):
                            for j in range(2):
                                hc = j * 4 + k
                                nc.tensor.matmul(
                                    sps[32 * j:32 * j + 1, :],
                                    v_col[:, hc:hc + 1], tanh_tiles[hc][:],
                                    start=(k == 0), stop=(k == 3),
                                    tile_position=(0, 32 * j))
                        srow = pwork.tile([1, 512], F32, tag="srow")
                        nc.vector.tensor_copy(srow[:], sps[32:33, :])
                        nc.vector.tensor_tensor(
                            out=row_b[0:1, n4 * 512:(n4 + 1) * 512],
                            in0=srow[:], in1=sps[0:1, :], op=ALU.add)

                    # P2: softmax on row_b (all on partition 0)
                    mrow = prow.tile([1, 4], F32, tag="mrow")
                    nc.vector.tensor_reduce(mrow[0:1, 0:1], row_b[:],
                                            axis=AX.X, op=ALU.max)
                    nc.vector.tensor_scalar_mul(mrow[0:1, 1:2], mrow[0:1, 0:1],
                                                -1.0)
                    nc.scalar.activation(row_b[:], row_b[:], ACT.Exp,
                                         bias=mrow[0:1, 1:2], scale=1.0,
                                         accum_out=mrow[0:1, 2:3])
                    nc.vector.reciprocal(mrow[0:1, 3:4], mrow[0:1, 2:3])
                    nc.vector.tensor_scalar_mul(row_b[:], row_b[:],
                                                mrow[0:1, 3:4])
                    nc.sync.dma_start(out=out_a.ap()[b:b + 1, :], in_=row_b[:])
                    a_bf = prow.tile([1, NK], BF, tag="abf")
                    nc.vector.tensor_copy(a_bf[:], row_b[:])
                    nc.sync.dma_start(out=a_nat[b:b + 1, :], in_=a_bf[:])

                # ---- P3: context = a @ ann (natural layout) ----
                # transpose a_nat into per-tile columns [128, BL]
                aT_sb = []
                for t in range(16):
                    tp = psmall.tile([128, BL], BF, tag="smallbf")
                    nc.tensor.transpose(tp[:],
                                        a_nat[0:BL, t * 128:(t + 1) * 128],
                                        ident_bf[0:BL, 0:BL])
                    at = pconst.tile([128, BL], BF, tag=f"aT{t}")
                    nc.vector.tensor_copy(at[:], tp[:])
                    aT_sb.append(at)
                ctx_nat = pconst.tile([BL, KEY], F32)
                for b in range(nbatch):
                    cps = pscore.tile([1, KEY], F32, tag="score")
                    for half in range(2):
                        an = pannN.tile([128, 8 * KEY], BF, tag="annN")
                        nc.sync.dma_start(
                            out=an[:],
                            in_=annN_h.ap()[b, :, half * 4096:(half + 1) * 4096])
                        for tt in range(8):
                            t = half * 8 + tt
                            nc.tensor.matmul(
                                cps[:], aT_sb[t][:, b:b + 1],
                                an[:, tt * KEY:(tt + 1) * KEY],
                                start=(t == 0), stop=(t == 15))
                    crow = pwork.tile([1, KEY], F32, tag="crow")
                    nc.vector.tensor_copy(crow[:], cps[:])
                    nc.sync.dma_start(out=ctx_nat[b:b + 1, :], in_=crow[:])

                if level >= 3:
                    # ctxT chunks [128, BL] for the GRU x.T stationary
                    ctx_bf = pconst.tile([BL, KEY], BF)
                    nc.vector.tensor_copy(ctx_bf[:], ctx_nat[:])
                    ctxT_bf = pconst.tile([128, 4 * BL], BF)
                    for t in range(4):
                        tp = psmall.tile([128, BL], BF, tag="smallbf")
                        nc.tensor.transpose(tp[:],
                                            ctx_bf[0:BL, t * 128:(t + 1) * 128],
                                            ident_bf[0:BL, 0:BL])
                        nc.vector.tensor_copy(
                            ctxT_bf[:, t * BL:(t + 1) * BL], tp[:])

                    # ---- P4: GRU ----
                    gi_sb = pconst.tile([BL, H3], BF)
                    gh_sb = pconst.tile([BL, H3], BF)
                    for which, w_h, b_h, g_sb in (
                        ("ih", wihT_h, bih_h, gi_sb),
                        ("hh", whhT_h, bhh_h, gh_sb),
                    ):
                        for rc in range(6):
                            bt = pstream.tile([1, 512], BF, tag="bg")
                            nc.sync.dma_start(
                                out=bt[:],
                                in_=b_h.ap()[0:1, rc * 512:(rc + 1) * 512])
                            wt = pwg.tile([128, 8 * 512], BF, tag="wg")
                            nc.sync.dma_start(out=wt[:], in_=w_h.ap()[rc])
                            gps = psmall.tile([BL, 512], F32, tag="small")
                            nc.tensor.matmul(gps[:], ones_bf[0:1, 0:BL], bt[:],
                                             start=True, stop=False)
                            for cc in range(8):
                                if which == "ih":
                                    lhsT = (embT_sb[:, cc * BL:(cc + 1) * BL]
                                            if cc < 4 else
                                            ctxT_bf[:, (cc - 4) * BL:(cc - 3) * BL])
                                else:
                                    lhsT = hidT_sb[:, cc * BL:(cc + 1) * BL]
                                nc.tensor.matmul(
                                    gps[:], lhsT,
                                    wt[:, cc * 512:(cc + 1) * 512],
                                    start=False, stop=(cc == 7))
                            nc.vector.tensor_copy(
                                g_sb[0:BL, rc * 512:(rc + 1) * 512], gps[:])

                    # gates
                    ta = pconst.tile([BL, HID], F32, tag="ta")
                    tb = pconst.tile([BL, HID], F32, tag="tb")
                    n_sb = pconst.tile([BL, HID], F32, tag="ngate")
                    h_sb = pconst.tile([BL, HID], F32, tag="hnew")
                    # r
                    nc.vector.tensor_tensor(out=ta[:], in0=gi_sb[0:BL, 0:HID],
                                            in1=gh_sb[0:BL, 0:HID], op=ALU.add)
                    nc.scalar.activation(tb[:], ta[:], ACT.Sigmoid)
                    # n = tanh(gi_n + r*gh_n)
                    nc.vector.tensor_tensor(out=ta[:], in0=tb[:],
                                            in1=gh_sb[0:BL, 2 * HID:H3],
                                            op=ALU.mult)
                    nc.vector.tensor_tensor(out=ta[:],
                                            in0=gi_sb[0:BL, 2 * HID:H3],
                                            in1=ta[:], op=ALU.add)
                    nc.scalar.activation(n_sb[:], ta[:], ACT.Tanh)
                    # z
                    nc.vector.tensor_tensor(out=ta[:],
                                            in0=gi_sb[0:BL, HID:2 * HID],
                                            in1=gh_sb[0:BL, HID:2 * HID],
                                            op=ALU.add)
                    nc.scalar.activation(tb[:], ta[:], ACT.Sigmoid)
                    # h = n + z*(Q - n)
                    nc.vector.tensor_tensor(out=ta[:], in0=hidn_sb[:],
                                            in1=n_sb[:], op=ALU.subtract)
                    nc.vector.tensor_tensor(out=ta[:], in0=tb[:], in1=ta[:],
                                            op=ALU.mult)
                    nc.vector.tensor_tensor(out=h_sb[:], in0=n_sb[:],
                                            in1=ta[:], op=ALU.add)
                    nc.sync.dma_start(out=out_h.ap()[:, :], in_=h_sb[:])

                if level >= 4:
                    # ---- P5: all-gather h ----
                    hb_in = pdram.tile([BL, HID], F32)
                    hb_out = pdram.tile([B, HID], F32)
                    nc.sync.dma_start(out=hb_in[:], in_=h_sb[:])
                    nc.gpsimd.collective_compute(
                        "AllGather", ALU.bypass,
                        replica_groups=[list(range(NCORES))],
                        ins=[hb_in.opt()], outs=[hb_out.opt()])
                    hall = pconst.tile([B, HID], F32)
                    nc.sync.dma_start(out=hall[:], in_=hb_out[:])
                    hallT = []
                    for hc in range(8):
                        tp = psmall.tile([128, B], F32, tag="small")
                        nc.tensor.transpose(tp[:],
                                            hall[:, hc * 128:(hc + 1) * 128],
                                            ident[0:B, 0:B])
                        ht = pconst.tile([128, B], BF, tag=f"hallT{hc}")
                        nc.vector.tensor_copy(ht[:], tp[:])
                        hallT.append(ht)

                if level >= 5:
                    # ---- P6: logits + local stats ----
                    logits = pconst.tile([B, VPAD], BF)
                    mx_sb = pconst.tile([B, 16], F32, tag="mx")
                    es_sb = pconst.tile([B, 16], F32, tag="es")
                    for vc in range(NVC):
                        bt = pstream.tile([1, 512], BF, tag="bg")
                        nc.sync.dma_start(
                            out=bt[:],
                            in_=bout_h.ap()[0:1, vc * 512:(vc + 1) * 512])
                        wot = pwot.tile([128, 8 * 512], BF, tag="wout")
                        nc.sync.dma_start(out=wot[:], in_=woutT_h.ap()[vc])
                        lp = ppre.tile([B, 512], F32, tag="pre")
                        nc.tensor.matmul(lp[:], ones_bf[0:1, 0:B], bt[:],
                                         start=True, stop=False)
                        for hc in range(8):
                            nc.tensor.matmul(
                                lp[:], hallT[hc][:],
                                wot[:, hc * 512:(hc + 1) * 512],
                                start=False, stop=(hc == 7))
                        nc.vector.tensor_copy(
                            logits[0:B, vc * 512:(vc + 1) * 512], lp[:])
                        nc.vector.tensor_reduce(mx_sb[0:B, vc:vc + 1], lp[:],
                                                axis=AX.X, op=ALU.max)
                    m_loc = pconst.tile([B, 4], F32, tag="mloc")
                    nc.vector.tensor_reduce(m_loc[0:B, 0:1], mx_sb[0:B, 0:NVC],
                                            axis=AX.X, op=ALU.max)
                    nc.vector.tensor_scalar_mul(m_loc[0:B, 1:2],
                                                m_loc[0:B, 0:1], -1.0)
                    for vc in range(NVC):
                        et = pwork.tile([B, 512], F32, tag="et")
                        nc.scalar.activation(
                            et[:], logits[0:B, vc * 512:(vc + 1) * 512],
                            ACT.Exp, bias=m_loc[0:B, 1:2], scale=1.0,
                            accum_out=es_sb[0:B, vc:vc + 1])
                    nc.vector.tensor_reduce(m_loc[0:B, 2:3], es_sb[0:B, 0:NVC],
                                            axis=AX.X, op=ALU.add)

                    # stats row: [1, 128] = [m_loc | s_loc] across batches
                    stats = pconst.tile([1, 2 * B], F32, tag="stats")
                    nc.sync.dma_start(out=stats[0:1, 0:B], in_=m_loc[0:B, 0:1])
                    nc.sync.dma_start(out=stats[0:1, B:2 * B],
                                      in_=m_loc[0:B, 2:3])
                    st_in = pdram.tile([1, 2 * B], F32)
                    st_out = pdram.tile([NCORES, 2 * B], F32)
                    nc.sync.dma_start(out=st_in[:], in_=stats[:])
                    nc.gpsimd.collective_compute(
                        "AllGather", ALU.bypass,
                        replica_groups=[list(range(NCORES))],
                        ins=[st_in.opt()], outs=[st_out.opt()])
                    statsall = pconst.tile([NCORES, 2 * B], F32, tag="statsall")
                    nc.sync.dma_start(out=statsall[:], in_=st_out[:])

                    # transpose to [B, NCORES]
                    mT = pconst.tile([B, NCORES], F32, tag="mT")
                    sT = pconst.tile([B, NCORES], F32, tag="sT")
                    tpm = psmall.tile([B, NCORES], F32, tag="small")
                    nc.tensor.transpose(tpm[:], statsall[0:NCORES, 0:B],
                                        ident[0:NCORES, 0:NCORES])
                    nc.vector.tensor_copy(mT[:], tpm[:])
                    tps = psmall.tile([B, NCORES], F32, tag="small")
                    nc.tensor.transpose(tps[:], statsall[0:NCORES, B:2 * B],
                                        ident[0:NCORES, 0:NCORES])
                    nc.vector.tensor_copy(sT[:], tps[:])

                    # global normalizer L = m_g + ln(sum_c s_c * exp(m_c - m_g))
                    fin = pconst.tile([B, 8], F32, tag="fin")
                    wrk = pconst.tile([B, NCORES], F32, tag="wrk")
                    nc.vector.tensor_reduce(fin[0:B, 0:1], mT[:], axis=AX.X,
                                            op=ALU.max)
                    nc.vector.tensor_scalar_sub(wrk[:], mT[:], fin[0:B, 0:1])
                    nc.scalar.activation(wrk[:], wrk[:], ACT.Exp)
                    nc.vector.tensor_tensor(out=wrk[:], in0=wrk[:], in1=sT[:],
                                            op=ALU.mult)
                    nc.vector.tensor_reduce(fin[0:B, 1:2], wrk[:], axis=AX.X,
                                            op=ALU.add)
                    nc.scalar.activation(fin[0:B, 2:3], fin[0:B, 1:2], ACT.Ln)
                    nc.vector.tensor_tensor(out=fin[0:B, 3:4],
                                            in0=fin[0:B, 0:1],
                                            in1=fin[0:B, 2:3], op=ALU.add)
                    # logp = logits - L, in f32 staging chunks
                    for vc in range(NVC):
                        lo = pwork.tile([B, 512], F32, tag="lpout")
                        nc.vector.tensor_scalar_sub(
                            lo[:], logits[0:B, vc * 512:(vc + 1) * 512],
                            fin[0:B, 3:4])
                        nc.sync.dma_start(
                            out=out_logp.ap()[:, vc * 512:(vc + 1) * 512],
                            in_=lo[:])

    nc.compile()
    return nc


_NC = None


def _get_nc():
    global _NC
    if _NC is None:
        _NC = _build(int(os.environ.get("KLEVEL", "9")))
    return _NC


def _shard(inputs):
    ids = np.asarray(inputs["input_ids"]).reshape(-1).astype(np.int64)
    emb = np.asarray(inputs["emb_table"], dtype=np.float32)[ids]      # [64, 512]
    hid = np.asarray(inputs["hidden"], dtype=np.float32)[0]           # [64, 1024]
    ann = np.asarray(inputs["annotations"], dtype=np.float32)         # [64, 2048, 512]

    wk_bf = np.asarray(inputs["Wk"], dtype=np.float32).astype(BF16)   # [512, 1024]
    wq_bf = np.asarray(inputs["Wq"], dtype=np.float32).astype(BF16)   # [1024, 1024]
    bk_col = np.ascontiguousarray(
        np.asarray(inputs["bk"], dtype=np.float32).reshape(HID, 1))
    v_bf = np.ascontiguousarray(
        np.asarray(inputs["v_attn"], dtype=np.float32).reshape(HID, 1)).astype(BF16)
    # tile-major [rc][128p][cc*512+j] = W.T[cc*128+p, rc*512+j]
    wihT = (np.asarray(inputs["W_ih"], dtype=np.float32).T.astype(BF16)
            .reshape(8, 128, 6, 512).transpose(2, 1, 0, 3)
            .reshape(6, 128, 8 * 512).copy())
    whhT = (np.asarray(inputs["W_hh"], dtype=np.float32).T.astype(BF16)
            .reshape(8, 128, 6, 512).transpose(2, 1, 0, 3)
            .reshape(6, 128, 8 * 512).copy())
    bih = np.asarray(inputs["b_ih"], dtype=np.float32).astype(BF16).reshape(1, H3)
    bhh = np.asarray(inputs["b_hh"], dtype=np.float32).astype(BF16).reshape(1, H3)
    wout = np.asarray(inputs["W_out"], dtype=np.float32)              # [50257, 1024]
    bout = np.asarray(inputs["b_out"], dtype=np.float32)

    in_maps = []
    for c in range(NCORES):
        sl = slice(c * BL, (c + 1) * BL)
        ann_bf = ann[sl].astype(BF16)
        annT = np.ascontiguousarray(ann_bf.transpose(0, 2, 1))
        # tile-major natural: [b][128p][t*512+k] = ann[b, t*128+p, k]
        annN = (ann_bf.reshape(BL, 16, 128, KEY).transpose(0, 2, 1, 3)
                .reshape(BL, 128, 16 * KEY).copy())
        hidT = np.ascontiguousarray(hid[sl].T).astype(BF16)           # [1024, 8]
        hidn = np.ascontiguousarray(hid[sl])                          # [8, 1024]
        embT = np.ascontiguousarray(emb[sl].T).astype(BF16)           # [512, 8]

        lo = c * VSH
        hi = min(lo + VSH, VOCAB)
        nrow = max(hi - lo, 0)
        woutT = np.zeros((HID, VPAD), dtype=BF16)
        if nrow > 0:
            woutT[:, :nrow] = wout[lo:hi].T.astype(BF16)
        # tile-major [vc][128p][hc*512+j] = woutT[hc*128+p, vc*512+j]
        woutT = (woutT.reshape(8, 128, NVC, 512).transpose(2, 1, 0, 3)
                 .reshape(NVC, 128, 8 * 512).copy())
        bout_sh = np.full((1, VPAD), NEG, dtype=np.float32)
        if nrow > 0:
            bout_sh[0, :nrow] = bout[lo:hi]
        in_maps.append({
            "annT": annT,
            "annN": annN,
            "hidT": hidT,
            "hidn": hidn,
            "embT": embT,
            "wk": wk_bf,
            "wq": wq_bf,
            "bk": bk_col,
            "vat": v_bf,
            "wihT": wihT,
            "whhT": whhT,
            "bih": bih,
            "bhh": bhh,
            "woutT": woutT,
            "bout": bout_sh.astype(BF16),
        })
    return in_maps


def kernel(trace=False, **inputs):
    nc = _get_nc()
    in_maps = _shard(inputs)
    res = bass_utils.run_bass_kernel_spmd(
        nc, in_maps, core_ids=list(range(NCORES)), trace=trace)
    results = res.results
    logp = np.concatenate(
        [results[c]["out_logp"][:, :VSH] for c in range(NCORES)], axis=1)
    logp = logp[:, :VOCAB].astype(np.float32)
    o = np.concatenate([results[c]["out_h"] for c in range(NCORES)], axis=0)
    a = np.concatenate([results[c]["out_a"] for c in range(NCORES)], axis=0)
    if trace:
        kernel.last_exec_time_ns = res.exec_time_ns
    return logp[None], o[None].astype(np.float32), a.astype(np.float32)


# revision 44
# speedup vs baseline: 1.4340x; 1.0186x over previous
"""Distributed Trainium2 kernel for a single-step attention decoder.

Bahdanau attention + single-step GRU + vocab projection with log-softmax,
SPMD across 8 NeuronCores:
  - batch-parallel (8 batches/core) for attention + GRU
  - vocab-sharded output projection with all-gathered max/sum normalizer

Self-contained: hardcodes all shapes; builds/compiles the Bass graph once and
caches it at module level.
"""

import os
import sys

sys.path.insert(0, "/opt/trn_rl_repo")

import numpy as np
import ml_dtypes

import concourse.bass as bass  # noqa: F401
import concourse.bacc as bacc
import concourse.mybir as mybir
import concourse.tile as tile
import concourse.bass_utils as bass_utils
from concourse import masks

BF16 = ml_dtypes.bfloat16
F32 = mybir.dt.float32
BF = mybir.dt.bfloat16

NCORES = 8
B, NK, HID, EMB, KEY, VOCAB = 64, 2048, 1024, 512, 512, 50257
BL = B // NCORES            # 8 batches per core
H3 = 3 * HID                # 3072
VSH = 6283                  # vocab rows per core (8*6283 = 50264 >= 50257)
VPAD = 6656                 # per-core padded to 13*512
NVC = VPAD // 512           # 13 vocab chunks
NEG = -1.0e9

ACT = mybir.ActivationFunctionType
ALU = mybir.AluOpType
AX = mybir.AxisListType


def _build(level=9):
    nc = bacc.Bacc("TRN2", target_bir_lowering=False, debug=False,
                   num_devices=NCORES)

    # ---- I/O ----
    # annT: transposed [key, nk] per batch; annN: tile-major natural
    # [b][128p][t*512+k] with t the nk-tile index
    annT_h = nc.dram_tensor("annT", [BL, KEY, NK], BF, kind="ExternalInput")
    annN_h = nc.dram_tensor("annN", [BL, 128, 16 * KEY], BF, kind="ExternalInput")
    hidT_h = nc.dram_tensor("hidT", [HID, BL], BF, kind="ExternalInput")
    hidn_h = nc.dram_tensor("hidn", [BL, HID], F32, kind="ExternalInput")
    embT_h = nc.dram_tensor("embT", [EMB, BL], BF, kind="ExternalInput")
    wk_h = nc.dram_tensor("wk", [KEY, HID], BF, kind="ExternalInput")
    wq_h = nc.dram_tensor("wq", [128, 8 * HID], BF, kind="ExternalInput")
    bk_h = nc.dram_tensor("bk", [HID, 1], F32, kind="ExternalInput")
    v_h = nc.dram_tensor("vat", [HID, 1], BF, kind="ExternalInput")
    # tile-major weight layouts: [chunk][128p][sub*512+j]
    wihT_h = nc.dram_tensor("wihT", [6, 128, 8 * 512], BF, kind="ExternalInput")
    whhT_h = nc.dram_tensor("whhT", [6, 128, 8 * 512], BF, kind="ExternalInput")
    bih_h = nc.dram_tensor("bih", [1, H3], BF, kind="ExternalInput")
    bhh_h = nc.dram_tensor("bhh", [1, H3], BF, kind="ExternalInput")
    woutT_h = nc.dram_tensor("woutT", [NVC, 128, 8 * 512], mybir.dt.float8e4,
                             kind="ExternalInput")
    bout_h = nc.dram_tensor("bout", [1, VPAD], BF, kind="ExternalInput")

    out_logp = nc.dram_tensor("out_logp", [B, VPAD], F32, kind="ExternalOutput")
    out_h = nc.dram_tensor("out_h", [BL, HID], F32, kind="ExternalOutput")
    out_a = nc.dram_tensor("out_a", [BL, NK], F32, kind="ExternalOutput")

    with tile.TileContext(nc, num_cores=NCORES) as tc:
        with (
            tc.tile_pool(name="pconst", bufs=1) as pconst,
            tc.tile_pool(name="pann", bufs=8) as pann,
            tc.tile_pool(name="ptanh", bufs=10) as ptanh,
            tc.tile_pool(name="pstream", bufs=3) as pstream,
            tc.tile_pool(name="pwg", bufs=2) as pwg,
            tc.tile_pool(name="pwot", bufs=2) as pwot,
            tc.tile_pool(name="pannN", bufs=2) as pannN,
            tc.tile_pool(name="pwork", bufs=2) as pwork,
            tc.tile_pool(name="prow", bufs=1) as prow,
            tc.tile_pool(name="pdram", bufs=1, space="DRAM") as pdram,
        ):
            # ---- constants / resident weights ----
            ident = pconst.tile([128, 128], F32)
            masks.make_identity(nc, ident[:])
            ident_bf = pconst.tile([128, 128], BF)
            masks.make_identity(nc, ident_bf[:])
            ones_bf = pconst.tile([1, 128], BF)
            nc.vector.memset(ones_bf[:], 1.0)

            wk_sb = []
            for kc in range(4):
                t = pconst.tile([128, HID], BF, tag=f"wk{kc}")
                nc.sync.dma_start(out=t[:], in_=wk_h.ap()[kc * 128:(kc + 1) * 128, :])
                wk_sb.append(t)

            v_col = pconst.tile([128, 8], BF)      # (p, hc)
            nc.sync.dma_start(
                out=v_col[:].rearrange("p (t o) -> p t o", o=1),
                in_=v_h.ap().rearrange("(t p) o -> p t o", p=128))
            bk_col = pconst.tile([128, 8], F32)
            nc.sync.dma_start(
                out=bk_col[:].rearrange("p (t o) -> p t o", o=1),
                in_=bk_h.ap().rearrange("(t p) o -> p t o", p=128))
            embT_sb = pconst.tile([128, 4 * BL], BF)   # (p, cc*8+b)
            nc.sync.dma_start(
                out=embT_sb[:].rearrange("p (t b) -> p t b", b=BL),
                in_=embT_h.ap().rearrange("(t p) b -> p t b", p=128))
            hidT_sb = pconst.tile([128, 8 * BL], BF)   # (p, ic*8+b)
            nc.sync.dma_start(
                out=hidT_sb[:].rearrange("p (t b) -> p t b", b=BL),
                in_=hidT_h.ap().rearrange("(t p) b -> p t b", p=128))
            hidn_sb = pconst.tile([BL, HID], F32)
            nc.sync.dma_start(out=hidn_sb[:], in_=hidn_h.ap())

            # ---- P0: biasT[:, hc*8+b] = (Q @ Wq).T + bk ----
            # scoped PSUM pool (8 banks) — closed before the main psum pools open
            biasT = pconst.tile([128, 8 * BL], F32)
            with tc.tile_pool(name="pq", bufs=8, space="PSUM") as pq:
                qps = [pq.tile([128, BL], F32, tag="qq", name=f"qq{i}")
                       for i in range(8)]
                wqt = pconst.tile([128, 8 * HID], BF, tag="wq")
                nc.sync.dma_start(out=wqt[:], in_=wq_h.ap()[:, :])
                for ic in range(8):
                    for hc in range(8):
                        nc.tensor.matmul(
                            qps[hc][:],
                            wqt[:, ic * HID + hc * 128:ic * HID + (hc + 1) * 128],
                            hidT_sb[:, ic * BL:(ic + 1) * BL],
                            start=(ic == 0), stop=(ic == 7))
                for hc in range(8):
                    nc.vector.tensor_scalar_add(
                        biasT[:, hc * BL:(hc + 1) * BL], qps[hc][:],
                        bk_col[:, hc:hc + 1])

            with (
                tc.tile_pool(name="ppre", bufs=2, space="PSUM") as ppre,
                tc.tile_pool(name="pscore", bufs=2, space="PSUM") as pscore,
                tc.tile_pool(name="pctx", bufs=1, space="PSUM") as pctx,
                tc.tile_pool(name="psmall", bufs=2, space="PSUM") as psmall,
                tc.tile_pool(name="psmallbf", bufs=1, space="PSUM") as psmallbf,
            ):
                # ---- P1-P3 per local batch b ----
                ctx_nat = pconst.tile([BL, KEY], F32)
                nbatch = BL if level >= 2 else 1
                for b in range(nbatch):
                    ann_t = []
                    for kc in range(4):
                        t = pann.tile([128, NK], BF, tag="ann")
                        nc.sync.dma_start(
                            out=t[:],
                            in_=annT_h.ap()[b, kc * 128:(kc + 1) * 128, :])
                        ann_t.append(t)
                    # prefetch natural-layout halves for the context matmul
                    an_halves = []
                    for half in range(2):
                        an = pannN.tile([128, 8 * KEY], BF, tag="annN")
                        nc.sync.dma_start(
                            out=an[:],
                            in_=annN_h.ap()[b, :, half * 4096:(half + 1) * 4096])
                        an_halves.append(an)

                    # scores row for this batch (partition 0)
                    row_b = prow.tile([1, NK], F32, tag="rowb")
                    for n4 in range(4):
                        tanh_tiles = []
                        for hc in range(8):
                            pre = ppre.tile([128, 512], F32, tag="pre")
                            for kc in range(4):
                                nc.tensor.matmul(
                                    pre[:], wk_sb[kc][:, hc * 128:(hc + 1) * 128],
                                    ann_t[kc][:, n4 * 512:(n4 + 1) * 512],
                                    start=(kc == 0), stop=(kc == 3))
                            th = ptanh.tile([128, 512], BF, tag="tanh")
                            nc.scalar.activation(
                                th[:], pre[:], ACT.Tanh,
                                bias=biasT[:, hc * BL + b:hc * BL + b + 1],
                                scale=1.0)
                            tanh_tiles.append(th)
                        # score reduce: two concurrent col-tile chains
                        sps = pscore.tile([64, 512], F32, tag="score")
                        for k in range(4):
                            for j in range(2):
                                hc = j * 4 + k
                                nc.tensor.matmul(
                                    sps[32 * j:32 * j + 1, :],
                                    v_col[:, hc:hc + 1], tanh_tiles[hc][:],
                                    start=(k == 0), stop=(k == 3),
                                    tile_position=(0, 32 * j))
                        srow = pwork.tile([1, 512], F32, tag="srow")
                        nc.vector.tensor_copy(srow[:], sps[32:33, :])
                        nc.vector.tensor_tensor(
                            out=row_b[0:1, n4 * 512:(n4 + 1) * 512],
                            in0=srow[:], in1=sps[0:1, :], op=ALU.add)

                    # P2: softmax on row_b (all on partition 0)
                    mrow = prow.tile([1, 4], F32, tag="mrow")
                    nc.vector.tensor_reduce(mrow[0:1, 0:1], row_b[:],
                                            axis=AX.X, op=ALU.max)
                    nc.vector.tensor_scalar_mul(mrow[0:1, 1:2], mrow[0:1, 0:1],
                                                -1.0)
                    nc.scalar.activation(row_b[:], row_b[:], ACT.Exp,
                                         bias=mrow[0:1, 1:2], scale=1.0,
                                         accum_out=mrow[0:1, 2:3])
                    nc.vector.reciprocal(mrow[0:1, 3:4], mrow[0:1, 2:3])
                    nc.vector.tensor_scalar_mul(row_b[:], row_b[:],
                                                mrow[0:1, 3:4])
                    nc.sync.dma_start(out=out_a.ap()[b:b + 1, :], in_=row_b[:])
                    a_bf = prow.tile([1, NK], BF, tag="abf")
                    nc.vector.tensor_copy(a_bf[:], row_b[:])

                    # P3: context = a @ ann (natural layout), interleaved per b
                    atp = psmallbf.tile([128, 32], BF, tag="smallbf")
                    for t in range(16):
                        nc.tensor.transpose(atp[:, 2 * t:2 * t + 1],
                                            a_bf[0:1, t * 128:(t + 1) * 128],
                                            ident_bf[0:1, 0:1])
                    aT = prow.tile([128, 16], BF, tag="aT")
                    nc.vector.tensor_copy(
                        aT[:].rearrange("p (t o) -> p t o", o=1),
                        atp[:].rearrange("p (t o) -> p t o", o=2)[:, :, 0:1])
                    cps = pctx.tile([1, KEY], F32, tag="cscore")
                    for t in range(16):
                        nc.tensor.matmul(
                            cps[:], aT[:, t:t + 1],
                            an_halves[t // 8][:, (t % 8) * KEY:(t % 8 + 1) * KEY],
                            start=(t == 0), stop=(t == 15))
                    crow = pwork.tile([1, KEY], F32, tag="crow")
                    nc.vector.tensor_copy(crow[:], cps[:])
                    nc.sync.dma_start(out=ctx_nat[b:b + 1, :], in_=crow[:])

                if level >= 3:
                    # ctxT chunks [128, BL] for the GRU x.T stationary
                    ctx_bf = pconst.tile([BL, KEY], BF)
                    nc.vector.tensor_copy(ctx_bf[:], ctx_nat[:])
                    ctxT_bf = pconst.tile([128, 4 * BL], BF)
                    for t in range(4):
                        tp = psmallbf.tile([128, 16], BF, tag="smallbf")
                        nc.tensor.transpose(tp[:, 0:BL],
                                            ctx_bf[0:BL, t * 128:(t + 1) * 128],
                                            ident_bf[0:BL, 0:BL])
                        nc.vector.tensor_copy(
                            ctxT_bf[:, t * BL:(t + 1) * BL], tp[:, 0:BL])

                    # ---- P4: GRU ----
                    gi_sb = pconst.tile([BL, H3], BF)
                    gh_sb = pconst.tile([BL, H3], BF)
                    for which, w_h, b_h, g_sb in (
                        ("ih", wihT_h, bih_h, gi_sb),
                        ("hh", whhT_h, bhh_h, gh_sb),
                    ):
                        for rc in range(6):
                            bt = pstream.tile([1, 512], BF, tag="bg")
                            nc.sync.dma_start(
                                out=bt[:],
                                in_=b_h.ap()[0:1, rc * 512:(rc + 1) * 512])
                            wt = pwg.tile([128, 8 * 512], BF, tag="wg")
                            nc.sync.dma_start(out=wt[:], in_=w_h.ap()[rc])
                            gps = psmall.tile([BL, 512], F32, tag="small")
                            nc.tensor.matmul(gps[:], ones_bf[0:1, 0:BL], bt[:],
                                             start=True, stop=False)
                            for cc in range(8):
                                if which == "ih":
                                    lhsT = (embT_sb[:, cc * BL:(cc + 1) * BL]
                                            if cc < 4 else
                                            ctxT_bf[:, (cc - 4) * BL:(cc - 3) * BL])
                                else:
                                    lhsT = hidT_sb[:, cc * BL:(cc + 1) * BL]
                                nc.tensor.matmul(
                                    gps[:], lhsT,
                                    wt[:, cc * 512:(cc + 1) * 512],
                                    start=False, stop=(cc == 7))
                            nc.vector.tensor_copy(
                                g_sb[0:BL, rc * 512:(rc + 1) * 512], gps[:])

                    # gates: rz = sigmoid(gi+gh) over r and z slices at once
                    trz = pconst.tile([BL, 2 * HID], F32, tag="trz")
                    rz = pconst.tile([BL, 2 * HID], F32, tag="rz")
                    ta = pconst.tile([BL, HID], F32, tag="ta")
                    n_sb = pconst.tile([BL, HID], F32, tag="ngate")
                    h_sb = pconst.tile([BL, HID], F32, tag="hnew")
                    nc.vector.tensor_tensor(out=trz[:],
                                            in0=gi_sb[0:BL, 0:2 * HID],
                                            in1=gh_sb[0:BL, 0:2 * HID],
                                            op=ALU.add)
                    nc.scalar.activation(rz[:], trz[:], ACT.Sigmoid)
                    # n = tanh(gi_n + r*gh_n)
                    nc.vector.tensor_tensor(out=ta[:], in0=rz[0:BL, 0:HID],
                                            in1=gh_sb[0:BL, 2 * HID:H3],
                                            op=ALU.mult)
                    nc.vector.tensor_tensor(out=ta[:],
                                            in0=gi_sb[0:BL, 2 * HID:H3],
                                            in1=ta[:], op=ALU.add)
                    nc.scalar.activation(n_sb[:], ta[:], ACT.Tanh)
                    # h = n + z*(Q - n)
                    nc.vector.tensor_tensor(out=ta[:], in0=hidn_sb[:],
                                            in1=n_sb[:], op=ALU.subtract)
                    nc.vector.tensor_tensor(out=ta[:],
                                            in0=rz[0:BL, HID:2 * HID],
                                            in1=ta[:], op=ALU.mult)
                    nc.vector.tensor_tensor(out=h_sb[:], in0=n_sb[:],
                                            in1=ta[:], op=ALU.add)
                    nc.sync.dma_start(out=out_h.ap()[:, :], in_=h_sb[:])

                if level >= 4:
                    # ---- P5: all-gather h ----
                    hb_in = pdram.tile([BL, HID], F32)
                    hb_out = pdram.tile([B, HID], F32)
                    nc.sync.dma_start(out=hb_in[:], in_=h_sb[:])
                    nc.gpsimd.collective_compute(
                        "AllGather", ALU.bypass,
                        replica_groups=[list(range(NCORES))],
                        ins=[hb_in.opt()], outs=[hb_out.opt()])
                    hall = pconst.tile([B, HID], F32)
                    nc.sync.dma_start(out=hall[:], in_=hb_out[:])
                    hallT = []
                    for hc in range(8):
                        tp = psmall.tile([128, B], F32, tag="small")
                        nc.tensor.transpose(tp[:],
                                            hall[:, hc * 128:(hc + 1) * 128],
                                            ident[0:B, 0:B])
                        ht = pconst.tile([128, B], mybir.dt.float8e4,
                                         tag=f"hallT{hc}")
                        nc.vector.tensor_copy(ht[:], tp[:])
                        hallT.append(ht)

                if level >= 5:
                    # ---- P6: logits + local stats ----
                    logits = pconst.tile([B, VPAD], BF)
                    mx_sb = pconst.tile([B, 16], F32, tag="mx")
                    es_sb = pconst.tile([B, 16], F32, tag="es")
                    for vc in range(NVC):
                        bt = pstream.tile([1, 512], BF, tag="bg")
                        nc.sync.dma_start(
                            out=bt[:],
                            in_=bout_h.ap()[0:1, vc * 512:(vc + 1) * 512])
                        wot = pwot.tile([128, 8 * 512], mybir.dt.float8e4,
                                        tag="wout")
                        nc.sync.dma_start(out=wot[:], in_=woutT_h.ap()[vc])
                        lp = ppre.tile([B, 512], F32, tag="pre")
                        nc.tensor.matmul(lp[:], ones_bf[0:1, 0:B], bt[:],
                                         start=True, stop=False)
                        for hc in range(8):
                            nc.tensor.matmul(
                                lp[:], hallT[hc][:],
                                wot[:, hc * 512:(hc + 1) * 512],
                                start=False, stop=(hc == 7))
                        nc.vector.tensor_copy(
                            logits[0:B, vc * 512:(vc + 1) * 512], lp[:])
                        nc.vector.tensor_reduce(mx_sb[0:B, vc:vc + 1], lp[:],
                                                axis=AX.X, op=ALU.max)
                    m_loc = pconst.tile([B, 4], F32, tag="mloc")
                    nc.vector.tensor_reduce(m_loc[0:B, 0:1], mx_sb[0:B, 0:NVC],
                                            axis=AX.X, op=ALU.max)
                    nc.vector.tensor_scalar_mul(m_loc[0:B, 1:2],
                                                m_loc[0:B, 0:1], -1.0)
                    for vc in range(NVC):
                        et = pwork.tile([B, 512], F32, tag="et")
                        nc.scalar.activation(
                            et[:], logits[0:B, vc * 512:(vc + 1) * 512],
                            ACT.Exp, bias=m_loc[0:B, 1:2], scale=1.0,
                            accum_out=es_sb[0:B, vc:vc + 1])
                    nc.vector.tensor_reduce(m_loc[0:B, 2:3], es_sb[0:B, 0:NVC],
                                            axis=AX.X, op=ALU.add)

                    # stats row: [1, 128] = [m_loc | s_loc] across batches
                    stats = pconst.tile([1, 2 * B], F32, tag="stats")
                    nc.sync.dma_start(out=stats[0:1, 0:B], in_=m_loc[0:B, 0:1])
                    nc.sync.dma_start(out=stats[0:1, B:2 * B],
                                      in_=m_loc[0:B, 2:3])
                    st_in = pdram.tile([1, 2 * B], F32)
                    st_out = pdram.tile([NCORES, 2 * B], F32)
                    nc.sync.dma_start(out=st_in[:], in_=stats[:])
                    nc.gpsimd.collective_compute(
                        "AllGather", ALU.bypass,
                        replica_groups=[list(range(NCORES))],
                        ins=[st_in.opt()], outs=[st_out.opt()])
                    statsall = pconst.tile([NCORES, 2 * B], F32, tag="statsall")
                    nc.sync.dma_start(out=statsall[:], in_=st_out[:])

                    # transpose to [B, NCORES]
                    mT = pconst.tile([B, NCORES], F32, tag="mT")
                    sT = pconst.tile([B, NCORES], F32, tag="sT")
                    tpm = psmall.tile([B, NCORES], F32, tag="small")
                    nc.tensor.transpose(tpm[:], statsall[0:NCORES, 0:B],
                                        ident[0:NCORES, 0:NCORES])
                    nc.vector.tensor_copy(mT[:], tpm[:])
                    tps = psmall.tile([B, NCORES], F32, tag="small")
                    nc.tensor.transpose(tps[:], statsall[0:NCORES, B:2 * B],
                                        ident[0:NCORES, 0:NCORES])
                    nc.vector.tensor_copy(sT[:], tps[:])

                    # global normalizer L = m_g + ln(sum_c s_c * exp(m_c - m_g))
                    fin = pconst.tile([B, 8], F32, tag="fin")
                    wrk = pconst.tile([B, NCORES], F32, tag="wrk")
                    nc.vector.tensor_reduce(fin[0:B, 0:1], mT[:], axis=AX.X,
                                            op=ALU.max)
                    nc.vector.tensor_scalar_sub(wrk[:], mT[:], fin[0:B, 0:1])
                    nc.scalar.activation(wrk[:], wrk[:], ACT.Exp)
                    nc.vector.tensor_tensor(out=wrk[:], in0=wrk[:], in1=sT[:],
                                            op=ALU.mult)
                    nc.vector.tensor_reduce(fin[0:B, 1:2], wrk[:], axis=AX.X,
                                            op=ALU.add)
                    nc.scalar.activation(fin[0:B, 2:3], fin[0:B, 1:2], ACT.Ln)
                    nc.vector.tensor_tensor(out=fin[0:B, 3:4],
                                            in0=fin[0:B, 0:1],
                                            in1=fin[0:B, 2:3], op=ALU.add)
                    # logp = logits - L, in f32 staging chunks
                    for vc in range(NVC):
                        lo = pwork.tile([B, 512], F32, tag="lpout")
                        nc.vector.tensor_scalar_sub(
                            lo[:], logits[0:B, vc * 512:(vc + 1) * 512],
                            fin[0:B, 3:4])
                        nc.sync.dma_start(
                            out=out_logp.ap()[:, vc * 512:(vc + 1) * 512],
                            in_=lo[:])

    nc.compile()
    return nc


_NC = None


def _get_nc():
    global _NC
    if _NC is None:
        _NC = _build(int(os.environ.get("KLEVEL", "9")))
    return _NC


def _shard(inputs):
    ids = np.asarray(inputs["input_ids"]).reshape(-1).astype(np.int64)
    emb = np.asarray(inputs["emb_table"], dtype=np.float32)[ids]      # [64, 512]
    hid = np.asarray(inputs["hidden"], dtype=np.float32)[0]           # [64, 1024]
    ann = np.asarray(inputs["annotations"], dtype=np.float32)         # [64, 2048, 512]

    wk_bf = np.asarray(inputs["Wk"], dtype=np.float32).astype(BF16)   # [512, 1024]
    # wq tile-major: [128p][ic*1024+h] = Wq[ic*128+p, h]
    wq_bf = (np.asarray(inputs["Wq"], dtype=np.float32).astype(BF16)
             .reshape(8, 128, HID).transpose(1, 0, 2).reshape(128, 8 * HID)
             .copy())
    bk_col = np.ascontiguousarray(
        np.asarray(inputs["bk"], dtype=np.float32).reshape(HID, 1))
    v_bf = np.ascontiguousarray(
        np.asarray(inputs["v_attn"], dtype=np.float32).reshape(HID, 1)).astype(BF16)
    # tile-major [rc][128p][cc*512+j] = W.T[cc*128+p, rc*512+j]
    wihT = (np.asarray(inputs["W_ih"], dtype=np.float32).T.astype(BF16)
            .reshape(8, 128, 6, 512).transpose(2, 1, 0, 3)
            .reshape(6, 128, 8 * 512).copy())
    whhT = (np.asarray(inputs["W_hh"], dtype=np.float32).T.astype(BF16)
            .reshape(8, 128, 6, 512).transpose(2, 1, 0, 3)
            .reshape(6, 128, 8 * 512).copy())
    bih = np.asarray(inputs["b_ih"], dtype=np.float32).astype(BF16).reshape(1, H3)
    bhh = np.asarray(inputs["b_hh"], dtype=np.float32).astype(BF16).reshape(1, H3)
    wout = np.asarray(inputs["W_out"], dtype=np.float32)              # [50257, 1024]
    bout = np.asarray(inputs["b_out"], dtype=np.float32)

    in_maps = []
    for c in range(NCORES):
        sl = slice(c * BL, (c + 1) * BL)
        ann_bf = ann[sl].astype(BF16)
        annT = np.ascontiguousarray(ann_bf.transpose(0, 2, 1))
        # tile-major natural: [b][128p][t*512+k] = ann[b, t*128+p, k]
        annN = (ann_bf.reshape(BL, 16, 128, KEY).transpose(0, 2, 1, 3)
                .reshape(BL, 128, 16 * KEY).copy())
        hidT = np.ascontiguousarray(hid[sl].T).astype(BF16)           # [1024, 8]
        hidn = np.ascontiguousarray(hid[sl])                          # [8, 1024]
        embT = np.ascontiguousarray(emb[sl].T).astype(BF16)           # [512, 8]

        lo = c * VSH
        hi = min(lo + VSH, VOCAB)
        nrow = max(hi - lo, 0)
        woutT = np.zeros((HID, VPAD), dtype=BF16)
        if nrow > 0:
            woutT[:, :nrow] = wout[lo:hi].T.astype(BF16)
        # tile-major [vc][128p][hc*512+j] = woutT[hc*128+p, vc*512+j], fp8
        woutT = (woutT.reshape(8, 128, NVC, 512).transpose(2, 1, 0, 3)
                 .reshape(NVC, 128, 8 * 512)
                 .astype(ml_dtypes.float8_e4m3))
        bout_sh = np.full((1, VPAD), NEG, dtype=np.float32)
        if nrow > 0:
            bout_sh[0, :nrow] = bout[lo:hi]
        in_maps.append({
            "annT": annT,
            "annN": annN,
            "hidT": hidT,
            "hidn": hidn,
            "embT": embT,
            "wk": wk_bf,
            "wq": wq_bf,
            "bk": bk_col,
            "vat": v_bf,
            "wihT": wihT,
            "whhT": whhT,
            "bih": bih,
            "bhh": bhh,
            "woutT": woutT,
            "bout": bout_sh.astype(BF16),
        })
    return in_maps


def kernel(trace=False, **inputs):
    nc = _get_nc()
    in_maps = _shard(inputs)
    res = bass_utils.run_bass_kernel_spmd(
        nc, in_maps, core_ids=list(range(NCORES)), trace=trace)
    results = res.results
    logp = np.concatenate(
        [results[c]["out_logp"][:, :VSH] for c in range(NCORES)], axis=1)
    logp = logp[:, :VOCAB].astype(np.float32)
    o = np.concatenate([results[c]["out_h"] for c in range(NCORES)], axis=0)
    a = np.concatenate([results[c]["out_a"] for c in range(NCORES)], axis=0)
    if trace:
        kernel.last_exec_time_ns = res.exec_time_ns
    return logp[None], o[None].astype(np.float32), a.astype(np.float32)


# revision 53
# speedup vs baseline: 1.8735x; 1.3065x over previous
"""Distributed Trainium2 kernel for a single-step attention decoder.

Bahdanau attention + single-step GRU + vocab projection with log-softmax,
SPMD across 8 NeuronCores:
  - batch-parallel (8 batches/core) for attention + GRU
  - vocab-sharded output projection with all-gathered max/sum normalizer

Self-contained: hardcodes all shapes; builds/compiles the Bass graph once and
caches it at module level.
"""

import os
import sys

sys.path.insert(0, "/opt/trn_rl_repo")

import numpy as np
import ml_dtypes

import concourse.bass as bass  # noqa: F401
import concourse.bacc as bacc
import concourse.mybir as mybir
import concourse.tile as tile
import concourse.bass_utils as bass_utils
from concourse import masks

BF16 = ml_dtypes.bfloat16
F32 = mybir.dt.float32
BF = mybir.dt.bfloat16

NCORES = 8
B, NK, HID, EMB, KEY, VOCAB = 64, 2048, 1024, 512, 512, 50257
BL = B // NCORES            # 8 batches per core
H3 = 3 * HID                # 3072
VSH = 6283                  # vocab rows per core (8*6283 = 50264 >= 50257)
VPAD = 6656                 # per-core padded to 13*512
NVC = VPAD // 512           # 13 vocab chunks
NEG = -1.0e9

ACT = mybir.ActivationFunctionType
ALU = mybir.AluOpType
AX = mybir.AxisListType


def _build(level=9):
    nc = bacc.Bacc("TRN2", target_bir_lowering=False, debug=False,
                   num_devices=NCORES)

    # ---- I/O ----
    # annT: DoubleRow-packed transposed fp8 [b][j][128p][i*NK+n]
    #   (k = j*256 + i*128 + p); annN: tile-major natural bf16
    FP8 = mybir.dt.float8e4
    annT_h = nc.dram_tensor("annT", [BL, 2, 128, 2 * NK], FP8, kind="ExternalInput")
    annN_h = nc.dram_tensor("annN", [BL, 128, 16 * KEY], BF, kind="ExternalInput")
    hidT_h = nc.dram_tensor("hidT", [HID, BL], BF, kind="ExternalInput")
    hidn_h = nc.dram_tensor("hidn", [BL, HID], F32, kind="ExternalInput")
    embT_h = nc.dram_tensor("embT", [EMB, BL], BF, kind="ExternalInput")
    # wk: DoubleRow-packed fp8, pre-scaled by 16: [j][128p][i*HID+h]
    wk_h = nc.dram_tensor("wk", [2, 128, 2 * HID], FP8, kind="ExternalInput")
    wq_h = nc.dram_tensor("wq", [128, 8 * HID], BF, kind="ExternalInput")
    bk_h = nc.dram_tensor("bk", [HID, 1], F32, kind="ExternalInput")
    v_h = nc.dram_tensor("vat", [HID, 1], BF, kind="ExternalInput")
    # tile-major weight layouts: [chunk][128p][sub*512+j]
    wihT_h = nc.dram_tensor("wihT", [6, 128, 8 * 512], BF, kind="ExternalInput")
    whhT_h = nc.dram_tensor("whhT", [6, 128, 8 * 512], BF, kind="ExternalInput")
    bih_h = nc.dram_tensor("bih", [1, H3], BF, kind="ExternalInput")
    bhh_h = nc.dram_tensor("bhh", [1, H3], BF, kind="ExternalInput")
    woutT_h = nc.dram_tensor("woutT", [NVC, 128, 8 * 512], mybir.dt.float8e4,
                             kind="ExternalInput")
    bout_h = nc.dram_tensor("bout", [1, VPAD], BF, kind="ExternalInput")

    out_logp = nc.dram_tensor("out_logp", [B, VPAD], F32, kind="ExternalOutput")
    out_h = nc.dram_tensor("out_h", [BL, HID], F32, kind="ExternalOutput")
    out_a = nc.dram_tensor("out_a", [BL, NK], F32, kind="ExternalOutput")

    with tile.TileContext(nc, num_cores=NCORES) as tc:
        with (
            tc.tile_pool(name="pconst", bufs=1) as pconst,
            tc.tile_pool(name="pann", bufs=4) as pann,
            tc.tile_pool(name="ptanh", bufs=10) as ptanh,
            tc.tile_pool(name="pstream", bufs=3) as pstream,
            tc.tile_pool(name="pwg", bufs=2) as pwg,
            tc.tile_pool(name="pwot", bufs=2) as pwot,
            tc.tile_pool(name="pannN", bufs=2) as pannN,
            tc.tile_pool(name="pwork", bufs=2) as pwork,
            tc.tile_pool(name="prow", bufs=1) as prow,
            tc.tile_pool(name="pdram", bufs=1, space="DRAM") as pdram,
        ):
            # ---- constants / resident weights ----
            ident = pconst.tile([128, 128], F32)
            masks.make_identity(nc, ident[:])
            ident_bf = pconst.tile([128, 128], BF)
            masks.make_identity(nc, ident_bf[:])
            ones_bf = pconst.tile([1, 128], BF)
            nc.vector.memset(ones_bf[:], 1.0)

            wk_sb = []
            for j in range(2):
                t = pconst.tile([128, 2 * HID], mybir.dt.float8e4, tag=f"wk{j}")
                nc.sync.dma_start(out=t[:], in_=wk_h.ap()[j])
                wk_sb.append(t)

            v_col = pconst.tile([128, 8], BF)      # (p, hc)
            nc.sync.dma_start(
                out=v_col[:].rearrange("p (t o) -> p t o", o=1),
                in_=v_h.ap().rearrange("(t p) o -> p t o", p=128))
            bk_col = pconst.tile([128, 8], F32)
            nc.sync.dma_start(
                out=bk_col[:].rearrange("p (t o) -> p t o", o=1),
                in_=bk_h.ap().rearrange("(t p) o -> p t o", p=128))
            embT_sb = pconst.tile([128, 4 * BL], BF)   # (p, cc*8+b)
            nc.sync.dma_start(
                out=embT_sb[:].rearrange("p (t b) -> p t b", b=BL),
                in_=embT_h.ap().rearrange("(t p) b -> p t b", p=128))
            hidT_sb = pconst.tile([128, 8 * BL], BF)   # (p, ic*8+b)
            nc.sync.dma_start(
                out=hidT_sb[:].rearrange("p (t b) -> p t b", b=BL),
                in_=hidT_h.ap().rearrange("(t p) b -> p t b", p=128))
            hidn_sb = pconst.tile([BL, HID], F32)
            nc.sync.dma_start(out=hidn_sb[:], in_=hidn_h.ap())

            # ---- P0: biasT[:, hc*8+b] = (Q @ Wq).T + bk ----
            # scoped PSUM pool (8 banks) — closed before the main psum pools open
            biasT = pconst.tile([128, 8 * BL], F32)
            with tc.tile_pool(name="pq", bufs=8, space="PSUM") as pq:
                qps = [pq.tile([128, BL], F32, tag="qq", name=f"qq{i}")
                       for i in range(8)]
                wqt = pconst.tile([128, 8 * HID], BF, tag="wq")
                nc.sync.dma_start(out=wqt[:], in_=wq_h.ap()[:, :])
                for ic in range(8):
                    for hc in range(8):
                        nc.tensor.matmul(
                            qps[hc][:],
                            wqt[:, ic * HID + hc * 128:ic * HID + (hc + 1) * 128],
                            hidT_sb[:, ic * BL:(ic + 1) * BL],
                            start=(ic == 0), stop=(ic == 7))
                for hc in range(8):
                    nc.vector.tensor_scalar_add(
                        biasT[:, hc * BL:(hc + 1) * BL], qps[hc][:],
                        bk_col[:, hc:hc + 1])

            with (
                tc.tile_pool(name="ppre", bufs=2, space="PSUM") as ppre,
                tc.tile_pool(name="pscore", bufs=2, space="PSUM") as pscore,
                tc.tile_pool(name="pctx", bufs=1, space="PSUM") as pctx,
                tc.tile_pool(name="psmall", bufs=2, space="PSUM") as psmall,
                tc.tile_pool(name="psmallbf", bufs=1, space="PSUM") as psmallbf,
            ):
                # ---- P1-P3 per local batch b ----
                ctx_nat = pconst.tile([BL, KEY], F32)
                nbatch = BL if level >= 2 else 1
                for b in range(nbatch):
                    ann_t = []
                    for j in range(2):
                        t = pann.tile([128, 2 * NK], mybir.dt.float8e4,
                                      tag="ann")
                        nc.sync.dma_start(out=t[:], in_=annT_h.ap()[b, j])
                        ann_t.append(t)
                    # prefetch natural-layout halves for the context matmul
                    an_halves = []
                    for half in range(2):
                        an = pannN.tile([128, 8 * KEY], BF, tag="annN")
                        nc.sync.dma_start(
                            out=an[:],
                            in_=annN_h.ap()[b, :, half * 4096:(half + 1) * 4096])
                        an_halves.append(an)

                    # scores row for this batch (partition 0)
                    row_b = prow.tile([1, NK], F32, tag="rowb")
                    for n4 in range(4):
                        tanh_tiles = []
                        for hc in range(8):
                            pre = ppre.tile([128, 512], F32, tag="pre")
                            for j in range(2):
                                lhsT = wk_sb[j][:].rearrange(
                                    "p (i h) -> p i h", i=2)[:, :, hc * 128:(hc + 1) * 128]
                                rhs = ann_t[j][:].rearrange(
                                    "p (i n) -> p i n", i=2)[:, :, n4 * 512:(n4 + 1) * 512]
                                nc.tensor.matmul(
                                    pre[:], lhsT, rhs,
                                    start=(j == 0), stop=(j == 1),
                                    perf_mode=mybir.MatmulPerfMode.DoubleRow)
                            th = ptanh.tile([128, 512], BF, tag="tanh")
                            # wk is pre-scaled by 16 for fp8; undo via scale
                            nc.scalar.activation(
                                th[:], pre[:], ACT.Tanh,
                                bias=biasT[:, hc * BL + b:hc * BL + b + 1],
                                scale=0.0625)
                            tanh_tiles.append(th)
                        # score reduce: two concurrent col-tile chains
                        sps = pscore.tile([64, 512], F32, tag="score")
                        for k in range(4):
                            for j in range(2):
                                hc = j * 4 + k
                                nc.tensor.matmul(
                                    sps[32 * j:32 * j + 1, :],
                                    v_col[:, hc:hc + 1], tanh_tiles[hc][:],
                                    start=(k == 0), stop=(k == 3),
                                    tile_position=(0, 32 * j))
                        srow = pwork.tile([1, 512], F32, tag="srow")
                        nc.vector.tensor_copy(srow[:], sps[32:33, :])
                        nc.vector.tensor_tensor(
                            out=row_b[0:1, n4 * 512:(n4 + 1) * 512],
                            in0=srow[:], in1=sps[0:1, :], op=ALU.add)

                    # P2: softmax on row_b (all on partition 0)
                    mrow = prow.tile([1, 4], F32, tag="mrow")
                    nc.vector.tensor_reduce(mrow[0:1, 0:1], row_b[:],
                                            axis=AX.X, op=ALU.max)
                    nc.vector.tensor_scalar_mul(mrow[0:1, 1:2], mrow[0:1, 0:1],
                                                -1.0)
                    nc.scalar.activation(row_b[:], row_b[:], ACT.Exp,
                                         bias=mrow[0:1, 1:2], scale=1.0,
                                         accum_out=mrow[0:1, 2:3])
                    nc.vector.reciprocal(mrow[0:1, 3:4], mrow[0:1, 2:3])
                    nc.vector.tensor_scalar_mul(row_b[:], row_b[:],
                                                mrow[0:1, 3:4])
                    nc.sync.dma_start(out=out_a.ap()[b:b + 1, :], in_=row_b[:])
                    a_bf = prow.tile([1, NK], BF, tag="abf")
                    nc.vector.tensor_copy(a_bf[:], row_b[:])

                    # P3: context = a @ ann (natural layout), interleaved per b
                    atp = psmallbf.tile([128, 32], BF, tag="smallbf")
                    for t in range(16):
                        nc.tensor.transpose(atp[:, 2 * t:2 * t + 1],
                                            a_bf[0:1, t * 128:(t + 1) * 128],
                                            ident_bf[0:1, 0:1])
                    aT = prow.tile([128, 16], BF, tag="aT")
                    nc.vector.tensor_copy(
                        aT[:].rearrange("p (t o) -> p t o", o=1),
                        atp[:].rearrange("p (t o) -> p t o", o=2)[:, :, 0:1])
                    cps = pctx.tile([1, KEY], F32, tag="cscore")
                    for t in range(16):
                        nc.tensor.matmul(
                            cps[:], aT[:, t:t + 1],
                            an_halves[t // 8][:, (t % 8) * KEY:(t % 8 + 1) * KEY],
                            start=(t == 0), stop=(t == 15))
                    crow = pwork.tile([1, KEY], F32, tag="crow")
                    nc.vector.tensor_copy(crow[:], cps[:])
                    nc.sync.dma_start(out=ctx_nat[b:b + 1, :], in_=crow[:])

                if level >= 3:
                    # ctxT chunks [128, BL] for the GRU x.T stationary
                    ctx_bf = pconst.tile([BL, KEY], BF)
                    nc.vector.tensor_copy(ctx_bf[:], ctx_nat[:])
                    ctxT_bf = pconst.tile([128, 4 * BL], BF)
                    for t in range(4):
                        tp = psmallbf.tile([128, 16], BF, tag="smallbf")
                        nc.tensor.transpose(tp[:, 0:BL],
                                            ctx_bf[0:BL, t * 128:(t + 1) * 128],
                                            ident_bf[0:BL, 0:BL])
                        nc.vector.tensor_copy(
                            ctxT_bf[:, t * BL:(t + 1) * BL], tp[:, 0:BL])

                    # ---- P4: GRU ----
                    gi_sb = pconst.tile([BL, H3], BF)
                    gh_sb = pconst.tile([BL, H3], BF)
                    for which, w_h, b_h, g_sb in (
                        ("ih", wihT_h, bih_h, gi_sb),
                        ("hh", whhT_h, bhh_h, gh_sb),
                    ):
                        for rc in range(6):
                            bt = pstream.tile([1, 512], BF, tag="bg")
                            nc.sync.dma_start(
                                out=bt[:],
                                in_=b_h.ap()[0:1, rc * 512:(rc + 1) * 512])
                            wt = pwg.tile([128, 8 * 512], BF, tag="wg")
                            nc.sync.dma_start(out=wt[:], in_=w_h.ap()[rc])
                            gps = psmall.tile([BL, 512], F32, tag="small")
                            nc.tensor.matmul(gps[:], ones_bf[0:1, 0:BL], bt[:],
                                             start=True, stop=False)
                            for cc in range(8):
                                if which == "ih":
                                    lhsT = (embT_sb[:, cc * BL:(cc + 1) * BL]
                                            if cc < 4 else
                                            ctxT_bf[:, (cc - 4) * BL:(cc - 3) * BL])
                                else:
                                    lhsT = hidT_sb[:, cc * BL:(cc + 1) * BL]
                                nc.tensor.matmul(
                                    gps[:], lhsT,
                                    wt[:, cc * 512:(cc + 1) * 512],
                                    start=False, stop=(cc == 7))
                            nc.vector.tensor_copy(
                                g_sb[0:BL, rc * 512:(rc + 1) * 512], gps[:])

                    # gates: rz = sigmoid(gi+gh) over r and z slices at once
                    trz = pconst.tile([BL, 2 * HID], F32, tag="trz")
                    rz = pconst.tile([BL, 2 * HID], F32, tag="rz")
                    ta = pconst.tile([BL, HID], F32, tag="ta")
                    n_sb = pconst.tile([BL, HID], F32, tag="ngate")
                    h_sb = pconst.tile([BL, HID], F32, tag="hnew")
                    nc.vector.tensor_tensor(out=trz[:],
                                            in0=gi_sb[0:BL, 0:2 * HID],
                                            in1=gh_sb[0:BL, 0:2 * HID],
                                            op=ALU.add)
                    nc.scalar.activation(rz[:], trz[:], ACT.Sigmoid)
                    # n = tanh(gi_n + r*gh_n)
                    nc.vector.tensor_tensor(out=ta[:], in0=rz[0:BL, 0:HID],
                                            in1=gh_sb[0:BL, 2 * HID:H3],
                                            op=ALU.mult)
                    nc.vector.tensor_tensor(out=ta[:],
                                            in0=gi_sb[0:BL, 2 * HID:H3],
                                            in1=ta[:], op=ALU.add)
                    nc.scalar.activation(n_sb[:], ta[:], ACT.Tanh)
                    # h = n + z*(Q - n)
                    nc.vector.tensor_tensor(out=ta[:], in0=hidn_sb[:],
                                            in1=n_sb[:], op=ALU.subtract)
                    nc.vector.tensor_tensor(out=ta[:],
                                            in0=rz[0:BL, HID:2 * HID],
                                            in1=ta[:], op=ALU.mult)
                    nc.vector.tensor_tensor(out=h_sb[:], in0=n_sb[:],
                                            in1=ta[:], op=ALU.add)
                    nc.sync.dma_start(out=out_h.ap()[:, :], in_=h_sb[:])

                if level >= 4:
                    # ---- P5: all-gather h ----
                    hb_in = pdram.tile([BL, HID], F32)
                    hb_out = pdram.tile([B, HID], F32)
                    nc.sync.dma_start(out=hb_in[:], in_=h_sb[:])
                    nc.gpsimd.collective_compute(
                        "AllGather", ALU.bypass,
                        replica_groups=[list(range(NCORES))],
                        ins=[hb_in.opt()], outs=[hb_out.opt()])
                    hall = pconst.tile([B, HID], F32)
                    nc.sync.dma_start(out=hall[:], in_=hb_out[:])
                    hallT = []
                    for hc in range(8):
                        tp = psmall.tile([128, B], F32, tag="small")
                        nc.tensor.transpose(tp[:],
                                            hall[:, hc * 128:(hc + 1) * 128],
                                            ident[0:B, 0:B])
                        ht = pconst.tile([128, B], mybir.dt.float8e4,
                                         tag=f"hallT{hc}")
                        nc.vector.tensor_copy(ht[:], tp[:])
                        hallT.append(ht)

                if level >= 5:
                    # ---- P6: logits + local stats ----
                    logits = pconst.tile([B, VPAD], BF)
                    mx_sb = pconst.tile([B, 16], F32, tag="mx")
                    es_sb = pconst.tile([B, 16], F32, tag="es")
                    for vc in range(NVC):
                        bt = pstream.tile([1, 512], BF, tag="bg")
                        nc.sync.dma_start(
                            out=bt[:],
                            in_=bout_h.ap()[0:1, vc * 512:(vc + 1) * 512])
                        wot = pwot.tile([128, 8 * 512], mybir.dt.float8e4,
                                        tag="wout")
                        nc.sync.dma_start(out=wot[:], in_=woutT_h.ap()[vc])
                        lp = ppre.tile([B, 512], F32, tag="pre")
                        nc.tensor.matmul(lp[:], ones_bf[0:1, 0:B], bt[:],
                                         start=True, stop=False)
                        for hc in range(8):
                            nc.tensor.matmul(
                                lp[:], hallT[hc][:],
                                wot[:, hc * 512:(hc + 1) * 512],
                                start=False, stop=(hc == 7))
                        nc.vector.tensor_copy(
                            logits[0:B, vc * 512:(vc + 1) * 512], lp[:])
                        nc.vector.tensor_reduce(
                            mx_sb[0:B, vc:vc + 1],
                            logits[0:B, vc * 512:(vc + 1) * 512],
                            axis=AX.X, op=ALU.max)
                    m_loc = pconst.tile([B, 4], F32, tag="mloc")
                    nc.vector.tensor_reduce(m_loc[0:B, 0:1], mx_sb[0:B, 0:NVC],
                                            axis=AX.X, op=ALU.max)
                    nc.vector.tensor_scalar_mul(m_loc[0:B, 1:2],
                                                m_loc[0:B, 0:1], -1.0)
                    for vc in range(NVC):
                        et = pwork.tile([B, 512], F32, tag="et")
                        nc.scalar.activation(
                            et[:], logits[0:B, vc * 512:(vc + 1) * 512],
                            ACT.Exp, bias=m_loc[0:B, 1:2], scale=1.0,
                            accum_out=es_sb[0:B, vc:vc + 1])
                    nc.vector.tensor_reduce(m_loc[0:B, 2:3], es_sb[0:B, 0:NVC],
                                            axis=AX.X, op=ALU.add)

                    # stats row: [1, 128] = [m_loc | s_loc] across batches
                    stats = pconst.tile([1, 2 * B], F32, tag="stats")
                    nc.sync.dma_start(out=stats[0:1, 0:B], in_=m_loc[0:B, 0:1])
                    nc.sync.dma_start(out=stats[0:1, B:2 * B],
                                      in_=m_loc[0:B, 2:3])
                    st_in = pdram.tile([1, 2 * B], F32)
                    st_out = pdram.tile([NCORES, 2 * B], F32)
                    nc.sync.dma_start(out=st_in[:], in_=stats[:])
                    nc.gpsimd.collective_compute(
                        "AllGather", ALU.bypass,
                        replica_groups=[list(range(NCORES))],
                        ins=[st_in.opt()], outs=[st_out.opt()])
                    statsall = pconst.tile([NCORES, 2 * B], F32, tag="statsall")
                    nc.sync.dma_start(out=statsall[:], in_=st_out[:])

                    # transpose to [B, NCORES]
                    mT = pconst.tile([B, NCORES], F32, tag="mT")
                    sT = pconst.tile([B, NCORES], F32, tag="sT")
                    tpm = psmall.tile([B, NCORES], F32, tag="small")
                    nc.tensor.transpose(tpm[:], statsall[0:NCORES, 0:B],
                                        ident[0:NCORES, 0:NCORES])
                    nc.vector.tensor_copy(mT[:], tpm[:])
                    tps = psmall.tile([B, NCORES], F32, tag="small")
                    nc.tensor.transpose(tps[:], statsall[0:NCORES, B:2 * B],
                                        ident[0:NCORES, 0:NCORES])
                    nc.vector.tensor_copy(sT[:], tps[:])

                    # global normalizer L = m_g + ln(sum_c s_c * exp(m_c - m_g))
                    fin = pconst.tile([B, 8], F32, tag="fin")
                    wrk = pconst.tile([B, NCORES], F32, tag="wrk")
                    nc.vector.tensor_reduce(fin[0:B, 0:1], mT[:], axis=AX.X,
                                            op=ALU.max)
                    nc.vector.tensor_scalar_sub(wrk[:], mT[:], fin[0:B, 0:1])
                    nc.scalar.activation(wrk[:], wrk[:], ACT.Exp)
                    nc.vector.tensor_tensor(out=wrk[:], in0=wrk[:], in1=sT[:],
                                            op=ALU.mult)
                    nc.vector.tensor_reduce(fin[0:B, 1:2], wrk[:], axis=AX.X,
                                            op=ALU.add)
                    nc.scalar.activation(fin[0:B, 2:3], fin[0:B, 1:2], ACT.Ln)
                    nc.vector.tensor_tensor(out=fin[0:B, 3:4],
                                            in0=fin[0:B, 0:1],
                                            in1=fin[0:B, 2:3], op=ALU.add)
                    # logp = logits - L, in f32 staging chunks
                    for vc in range(NVC):
                        lo = pwork.tile([B, 512], F32, tag="lpout")
                        nc.vector.tensor_scalar_sub(
                            lo[:], logits[0:B, vc * 512:(vc + 1) * 512],
                            fin[0:B, 3:4])
                        nc.sync.dma_start(
                            out=out_logp.ap()[:, vc * 512:(vc + 1) * 512],
                            in_=lo[:])

    nc.compile()
    return nc


_NC = None


def _get_nc():
    global _NC
    if _NC is None:
        _NC = _build(int(os.environ.get("KLEVEL", "9")))
    return _NC


def _shard(inputs):
    ids = np.asarray(inputs["input_ids"]).reshape(-1).astype(np.int64)
    emb = np.asarray(inputs["emb_table"], dtype=np.float32)[ids]      # [64, 512]
    hid = np.asarray(inputs["hidden"], dtype=np.float32)[0]           # [64, 1024]
    ann = np.asarray(inputs["annotations"], dtype=np.float32)         # [64, 2048, 512]

    FP8NP = ml_dtypes.float8_e4m3
    # wk DoubleRow-packed fp8 scaled by 16: [j][p][i*HID+h]
    wk_bf = ((np.asarray(inputs["Wk"], dtype=np.float32) * 16.0)
             .reshape(2, 2, 128, HID).transpose(0, 2, 1, 3)
             .reshape(2, 128, 2 * HID).astype(FP8NP))
    # wq tile-major: [128p][ic*1024+h] = Wq[ic*128+p, h]
    wq_bf = (np.asarray(inputs["Wq"], dtype=np.float32).astype(BF16)
             .reshape(8, 128, HID).transpose(1, 0, 2).reshape(128, 8 * HID)
             .copy())
    bk_col = np.ascontiguousarray(
        np.asarray(inputs["bk"], dtype=np.float32).reshape(HID, 1))
    v_bf = np.ascontiguousarray(
        np.asarray(inputs["v_attn"], dtype=np.float32).reshape(HID, 1)).astype(BF16)
    # tile-major [rc][128p][cc*512+j] = W.T[cc*128+p, rc*512+j]
    wihT = (np.asarray(inputs["W_ih"], dtype=np.float32).T.astype(BF16)
            .reshape(8, 128, 6, 512).transpose(2, 1, 0, 3)
            .reshape(6, 128, 8 * 512).copy())
    whhT = (np.asarray(inputs["W_hh"], dtype=np.float32).T.astype(BF16)
            .reshape(8, 128, 6, 512).transpose(2, 1, 0, 3)
            .reshape(6, 128, 8 * 512).copy())
    bih = np.asarray(inputs["b_ih"], dtype=np.float32).astype(BF16).reshape(1, H3)
    bhh = np.asarray(inputs["b_hh"], dtype=np.float32).astype(BF16).reshape(1, H3)
    wout = np.asarray(inputs["W_out"], dtype=np.float32)              # [50257, 1024]
    bout = np.asarray(inputs["b_out"], dtype=np.float32)

    in_maps = []
    for c in range(NCORES):
        sl = slice(c * BL, (c + 1) * BL)
        ann_bf = ann[sl].astype(BF16)
        # DoubleRow-packed fp8 transposed: [b][j][p][i*NK+n]
        annT = (ann[sl].transpose(0, 2, 1)
                .reshape(BL, 2, 2, 128, NK).transpose(0, 1, 3, 2, 4)
                .reshape(BL, 2, 128, 2 * NK).astype(FP8NP))
        # tile-major natural: [b][128p][t*512+k] = ann[b, t*128+p, k]
        annN = (ann_bf.reshape(BL, 16, 128, KEY).transpose(0, 2, 1, 3)
                .reshape(BL, 128, 16 * KEY).copy())
        hidT = np.ascontiguousarray(hid[sl].T).astype(BF16)           # [1024, 8]
        hidn = np.ascontiguousarray(hid[sl])                          # [8, 1024]
        embT = np.ascontiguousarray(emb[sl].T).astype(BF16)           # [512, 8]

        lo = c * VSH
        hi = min(lo + VSH, VOCAB)
        nrow = max(hi - lo, 0)
        woutT = np.zeros((HID, VPAD), dtype=BF16)
        if nrow > 0:
            woutT[:, :nrow] = wout[lo:hi].T.astype(BF16)
        # tile-major [vc][128p][hc*512+j] = woutT[hc*128+p, vc*512+j], fp8
        woutT = (woutT.reshape(8, 128, NVC, 512).transpose(2, 1, 0, 3)
                 .reshape(NVC, 128, 8 * 512)
                 .astype(ml_dtypes.float8_e4m3))
        bout_sh = np.full((1, VPAD), NEG, dtype=np.float32)
        if nrow > 0:
            bout_sh[0, :nrow] = bout[lo:hi]
        in_maps.append({
            "annT": annT,
            "annN": annN,
            "hidT": hidT,
            "hidn": hidn,
            "embT": embT,
            "wk": wk_bf,
            "wq": wq_bf,
            "bk": bk_col,
            "vat": v_bf,
            "wihT": wihT,
            "whhT": whhT,
            "bih": bih,
            "bhh": bhh,
            "woutT": woutT,
            "bout": bout_sh.astype(BF16),
        })
    return in_maps


def kernel(trace=False, **inputs):
    nc = _get_nc()
    in_maps = _shard(inputs)
    res = bass_utils.run_bass_kernel_spmd(
        nc, in_maps, core_ids=list(range(NCORES)), trace=trace)
    results = res.results
    logp = np.concatenate(
        [results[c]["out_logp"][:, :VSH] for c in range(NCORES)], axis=1)
    logp = logp[:, :VOCAB].astype(np.float32)
    o = np.concatenate([results[c]["out_h"] for c in range(NCORES)], axis=0)
    a = np.concatenate([results[c]["out_a"] for c in range(NCORES)], axis=0)
    if trace:
        kernel.last_exec_time_ns = res.exec_time_ns
    return logp[None], o[None].astype(np.float32), a.astype(np.float32)
